# revision 1
# baseline (speedup 1.0000x reference)
import numpy as np

SQ2 = 2.0 ** 0.5
H = W = 512
HH = 256  # H//2
NCH = 8
NCORES = 8

_cache = {}


def _filters():
    hh = np.array([0.037828455506995, -0.02384946501938, -0.11062440441842, 0.37740285561265], np.float32)
    h = np.concatenate([hh, [np.float32(0.8526986790094)], hh[::-1]]).astype(np.float32)
    gg = np.array([-0.064538882628938, -0.040689417609558, 0.41809227322221], np.float32)
    g = np.concatenate([gg, [np.float32(0.78848561640566)], gg[::-1]]).astype(np.float32)
    v = np.array([0.63, -0.193, 0.0972, -0.0526, 0.0272, -0.0144], np.float32)
    f = np.concatenate([v[::-1], v]).astype(np.float32)
    f[::2] = -f[::2]
    return h, g, f


def _host_mats(h, g, f):
    # BhP [520,256]: fused 9-tap h conv + 2x avgpool (rows): out[i] = 0.5*(L[2i]+L[2i+1])
    BhP = np.zeros((520, 256), np.float32)
    for i in range(256):
        for p in (0, 1):
            r = 2 * i + p
            for u in range(9):
                BhP[r + u, i] += 0.5 * h[u]
    # GU [260,512]: upsample cA (pad2) + 7-tap g conv: Mrow[r] = sum_u g[u] D[r+u-3], D[2a']=cA[a']
    GU = np.zeros((260, 512), np.float32)
    for r in range(512):
        for u in range(7):
            al = r + u - 3
            if al % 2 == 0:
                a = al // 2 + 2  # cAp row (pad 2)
                GU[a, r] += g[u]
    GU8 = (8.0 * GU).astype(np.float32)
    # Bf256 [267,256], Bf512 [523,512]: 12-tap f bands
    Bf256 = np.zeros((267, 256), np.float32)
    for o in range(256):
        for u in range(12):
            Bf256[o + u, o] = f[u]
    Bf512 = np.zeros((523, 512), np.float32)
    for o in range(512):
        for u in range(12):
            Bf512[o + u, o] = f[u]
    ident = np.eye(128, dtype=np.float32)
    return {"BhP": BhP, "GU8": GU8, "GU": GU, "Bf256": Bf256, "Bf512": Bf512, "ident": ident}


def _build_nc():
    import concourse.bass as bass
    import concourse.bacc as bacc
    import concourse.mybir as mybir
    from concourse import tile

    FP = mybir.dt.float32
    nc = bacc.Bacc("TRN2", target_bir_lowering=False, debug=False, num_devices=NCORES)
    AP = bass.AP

    x_h = nc.dram_tensor("x", [NCH, H, W], FP, kind="ExternalInput")
    mat_hs = {}
    for nm, shp in [("BhP", (520, 256)), ("GU8", (260, 512)), ("GU", (260, 512)),
                    ("Bf256", (267, 256)), ("Bf512", (523, 512)), ("ident", (128, 128))]:
        mat_hs[nm] = nc.dram_tensor(nm, list(shp), FP, kind="ExternalInput")
    outs = {}
    for nm in ["out_c", "out_e1lo", "out_e0lo", "out_e1hi", "out_e0hi"]:
        outs[nm] = nc.dram_tensor(nm, [NCH, HH, HH], FP, kind="ExternalOutput")

    # internal DRAM
    WET = 832
    Xd = nc.dram_tensor("Xd", [H, W], FP, kind="Internal")
    cAd = nc.dram_tensor("cAd", [HH, HH], FP, kind="Internal")
    Md = nc.dram_tensor("Md", [H, W], FP, kind="Internal")
    ET2 = nc.dram_tensor("ET2", [WET, WET], FP, kind="Internal")       # DsumT tiled, origin 20,20
    EXX = nc.dram_tensor("EXX", [560, WET], FP, kind="Internal")       # XXs qext, row A+16, col B+280
    E1 = nc.dram_tensor("E1", [512, 512], FP, kind="Internal")         # S1 qext: vstack(rot256(S1), S1)
    E2 = nc.dram_tensor("E2", [512, 512], FP, kind="Internal")
    X0d = nc.dram_tensor("X0d", [HH, W], FP, kind="Internal")
    E0d = nc.dram_tensor("E0d", [HH, HH], FP, kind="Internal")

    def dram_ap(hd, off, dims):
        return AP(hd, off, [list(d) for d in dims])

    with tile.TileContext(nc) as tc:
        import contextlib
        ctx = contextlib.ExitStack()
        with ctx:
            cpool = ctx.enter_context(tc.tile_pool(name="consts", bufs=1))
            work = ctx.enter_context(tc.tile_pool(name="work", bufs=2))
            keep = ctx.enter_context(tc.tile_pool(name="keep", bufs=1))
            tmp = ctx.enter_context(tc.tile_pool(name="tmp", bufs=1))
            outp = ctx.enter_context(tc.tile_pool(name="outp", bufs=4))
            psum = ctx.enter_context(tc.tile_pool(name="ps", bufs=4, space="PSUM"))

            # ---- load const matrices as K-chunked [<=128, M] tiles ----
            mats = {}
            for nm, (K, M) in [("BhP", (520, 256)), ("GU8", (260, 512)), ("GU", (260, 512)),
                               ("Bf256", (267, 256)), ("Bf512", (523, 512))]:
                tl = []
                for k0 in range(0, K, 128):
                    kk = min(128, K - k0)
                    t = cpool.tile([kk, M], FP, tag=f"m_{nm}_{k0}")
                    nc.sync.dma_start(t[:], mat_hs[nm].ap()[k0:k0 + kk, :])
                    tl.append((k0, kk, t))
                mats[nm] = tl
            identt = cpool.tile([128, 128], FP, tag="ident")
            nc.sync.dma_start(identt[:], mat_hs["ident"].ap()[:, :])

            def transpose_tiles(src_tiles, R, C, pool, tag):
                # src_tiles: list (r0, nr, tile[nr, C]) covering [R, C] -> returns tiles of [C, R]
                outt = []
                for c0 in range(0, C, 128):
                    cw = min(128, C - c0)
                    t = pool.tile([cw, R], FP, tag=f"{tag}_{c0}")
                    for (r0, nr, st) in src_tiles:
                        ps = psum.tile([cw, nr], FP, tag="ps")
                        nc.tensor.transpose(ps[:, :], st[:, c0:c0 + cw], identt[:nr, :nr])
                        nc.vector.tensor_copy(t[:, r0:r0 + nr], ps[:, :])
                    outt.append((c0, cw, t))
                return outt

            def band_pass(in_tiles, mat_tiles, M, N, pool, tag):
                # out[m, n] = sum_k mat[k, m] * in[k, n]; in_tiles chunked at 128 rows
                outt = []
                for m0 in range(0, M, 128):
                    mw = min(128, M - m0)
                    t = pool.tile([mw, N], FP, tag=f"{tag}_{m0}")
                    for n0 in range(0, N, 512):
                        nw = min(512, N - n0)
                        ps = psum.tile([mw, nw], FP, tag="ps")
                        nk = len(in_tiles)
                        for ki, ((k0, kk, it), (mk0, mkk, mt)) in enumerate(zip(in_tiles, mat_tiles)):
                            assert k0 == mk0 and kk == mkk
                            nc.tensor.matmul(ps[:, :], mt[:, m0:m0 + mw], it[:, n0:n0 + nw],
                                             start=(ki == 0), stop=(ki == nk - 1))
                        nc.vector.tensor_copy(t[:, n0:n0 + nw], ps[:, :])
                    outt.append((m0, mw, t))
                return outt

            def conv2(in_tiles, Rp, Cp, mrow, mcol, Mr, Mc, pool, tag):
                # in_tiles cover padded [Rp, Cp]; returns result [Mr, Mc] tiles (normal orientation)
                p1 = band_pass(in_tiles, mats[mrow], Mr, Cp, tmp, "cvp1")
                p1t = transpose_tiles(p1, Mr, Cp, tmp, "cvt1")
                p2 = band_pass(p1t, mats[mcol], Mc, Mr, tmp, "cvp2")  # [Mc, Mr]
                return transpose_tiles(p2, Mc, Mr, pool, tag)

            def pad_per_from_dram(hd, R, C, ru, cl, Rp, Cp, pool, tag, qper=False):
                # build padded tiles [Rp, Cp] from DRAM map [R, C]; padded (k,c) -> src row (k-ru)%R col (c-cl)%C
                # qper: rows outside [0,R) additionally rotate cols by C//2
                tiles = []
                for k0 in range(0, Rp, 128):
                    kk = min(128, Rp - k0)
                    t = tmp.tile([kk, Cp], FP, tag=f"pad_{k0}")
                    # split rows into runs with same wrap-band
                    k = k0
                    while k < k0 + kk:
                        a = k - ru  # source row index (unwrapped)
                        band = 0 if 0 <= a < R else (-1 if a < 0 else 1)
                        # run length until band changes or tile ends
                        if band == -1:
                            run = min(k0 + kk - k, -a)
                        elif band == 0:
                            run = min(k0 + kk - k, R - a)
                        else:
                            run = k0 + kk - k
                        sr = a % R
                        rot = (C // 2) if (qper and band != 0) else 0
                        # cols: padded c -> src (c - cl + rot) % C ; emit contiguous segments
                        c = 0
                        while c < Cp:
                            sc = (c - cl + rot) % C
                            seg = min(Cp - c, C - sc)
                            nc.sync.dma_start(
                                t[k - k0:k - k0 + run, c:c + seg],
                                dram_ap(hd, sr * C + sc, [[C, run], [1, seg]]))
                            c += seg
                        k += run
                    tiles.append((k0, kk, t))
                return tiles

            # ================= stage 1: X = channel sum of x =================
            for r0 in range(0, H, 128):
                xa = work.tile([128, W], FP, tag="xsumA")
                xb = work.tile([128, W], FP, tag="xsumB")
                acc = work.tile([128, W], FP, tag="xsumAcc")
                nc.sync.dma_start(xa[:], x_h.ap()[0, r0:r0 + 128, :])
                nc.sync.dma_start(xb[:], x_h.ap()[1, r0:r0 + 128, :])
                nc.vector.scalar_tensor_tensor(acc[:], xa[:], 1.0, xb[:],
                                               mybir.AluOpType.mult, mybir.AluOpType.add)
                for ch in range(2, NCH):
                    xc = work.tile([128, W], FP, tag="xsumC")
                    nc.sync.dma_start(xc[:], x_h.ap()[ch, r0:r0 + 128, :])
                    nc.vector.scalar_tensor_tensor(acc[:], xc[:], 1.0, acc[:],
                                                   mybir.AluOpType.mult, mybir.AluOpType.add)
                nc.sync.dma_start(Xd.ap()[r0:r0 + 128, :], acc[:])

            # ================= stage 2: cA = pool(conv_h(X)) =================
            Xp = pad_per_from_dram(Xd, H, W, 4, 4, 520, 520, work, "Xp")
            cAt = conv2(Xp, 520, 520, "BhP", "BhP", 256, 256, keep, "cA")
            for (r0, nr, t) in cAt:
                nc.sync.dma_start(cAd.ap()[r0:r0 + nr, :], t[:])
                for ch in range(NCH):
                    nc.sync.dma_start(outs["out_c"].ap()[ch, r0:r0 + nr, :], t[:])

            # ================= stage 3: M = 8*conv_g(dup(cA)) =================
            cAp = pad_per_from_dram(cAd, HH, HH, 2, 2, 260, 260, work, "cAp")
            Mt = conv2(cAp, 260, 260, "GU8", "GU", H, W, keep, "M")
            for (r0, nr, t) in Mt:
                nc.sync.dma_start(Md.ap()[r0:r0 + nr, :], t[:])

            # ================= stage 4: Dsum = X - 8M; DsumT -> ET2 =================
            Dst = []
            for (r0, nr, mt) in Mt:
                xt = work.tile([128, W], FP, tag="Xrd")
                nc.sync.dma_start(xt[:], Xd.ap()[r0:r0 + nr, :])
                d = tmp.tile([128, W], FP, tag=f"Ds_{r0}")
                nc.vector.scalar_tensor_tensor(d[:], mt[:], -8.0, xt[:],
                                               mybir.AluOpType.mult, mybir.AluOpType.add)
                Dst.append((r0, nr, d))
            DsT = transpose_tiles(Dst, H, W, tmp, "DsT")
            # ET2[u,v] = DsumT[(u-20)%512, (v-20)%512], [832,832]
            for (r0, nr, t) in DsT:
                for (du, su, cnt_u) in [(0, 492, 20), (20, 0, 512), (532, 0, 300)]:
                    lo = max(r0, su)
                    hi = min(r0 + nr, su + cnt_u)
                    if lo >= hi:
                        continue
                    u0 = du + (lo - su)
                    for (dv, sv, cnt_v) in [(0, 492, 20), (20, 0, 512), (532, 0, 300)]:
                        nc.sync.dma_start(
                            dram_ap(ET2, u0 * WET + dv, [[WET, hi - lo], [1, cnt_v]]),
                            t[lo - r0:hi - r0, sv:sv + cnt_v])

            # gather helper: (a,b) map from ET2: val = Dsum[(a-b)%512,(a+b+delta)%512]
            # ET2[u,v]=Dsum[(v-20)%512,(u-20)%512] -> u = a+b+delta+20, v = a-b+532  (532%512==20)
            def gather_ab(delta, a0, b0, nr, ncol, dst):
                u0 = a0 + b0 + delta + 20
                v0 = a0 - b0 + 532
                nc.sync.dma_start(dst, dram_ap(ET2, u0 * WET + v0,
                                               [[WET + 1, nr], [WET - 1, ncol]]))

            # ================= stage 5: S1 = conv_f_qper(P1s, shift11) =================
            # padded P1s: rows a=k-6 in [-6,261), cols b=c-6 in [-6,517)
            P1p = []
            for k0 in range(0, 267, 128):
                kk = min(128, 267 - k0)
                t = work.tile([kk, 523], FP, tag=f"P1p_{k0}")
                gather_ab(1, k0 - 6, -6, kk, 523, t[:])
                P1p.append((k0, kk, t))
            S1t = conv2(P1p, 267, 523, "Bf256", "Bf512", HH, W, keep, "S1")
            # E1 = vstack(rot256(S1), S1)
            for (r0, nr, t) in S1t:
                nc.sync.dma_start(E1.ap()[r0:r0 + nr, 0:256], t[:, 256:512])
                nc.sync.dma_start(E1.ap()[r0:r0 + nr, 256:512], t[:, 0:256])
                nc.sync.dma_start(E1.ap()[256 + r0:256 + r0 + nr, :], t[:])

            # X0s = (P0s - 8*S1)/sqrt2
            X0t = []
            for (r0, nr, s1) in S1t:
                p0 = work.tile([nr, W], FP, tag=f"P0g_{r0}")
                gather_ab(0, r0, 0, nr, W, p0[:])
                x0 = keep.tile([nr, W], FP, tag=f"X0_{r0}")
                nc.vector.scalar_tensor_tensor(x0[:], s1[:], -8.0, p0[:],
                                               mybir.AluOpType.mult, mybir.AluOpType.add)
                nc.vector.tensor_scalar_mul(x0[:], x0[:], 1.0 / SQ2)
                nc.sync.dma_start(X0d.ap()[r0:r0 + nr, :], x0[:])
                X0t.append((r0, nr, x0))

            # ================= stage 6: S2 = conv_f_qper(X0s, shift00) =================
            X0p = pad_per_from_dram(X0d, HH, W, 5, 5, 267, 523, work, "X0p", qper=True)
            S2t = conv2(X0p, 267, 523, "Bf256", "Bf512", HH, W, keep, "S2")
            for (r0, nr, t) in S2t:
                nc.sync.dma_start(E2.ap()[r0:r0 + nr, 0:256], t[:, 256:512])
                nc.sync.dma_start(E2.ap()[r0:r0 + nr, 256:512], t[:, 0:256])
                nc.sync.dma_start(E2.ap()[256 + r0:256 + r0 + nr, :], t[:])

            # XXs = X0s - sqrt2*P1s - 8*S2 ; EXX[p,q]=XXs[(p-16)%256, ((q-280)+256*floor((p-16)/256))%512]
            for (r0, nr, s2) in S2t:
                p1 = work.tile([nr, W], FP, tag=f"P1g_{r0}")
                gather_ab(1, r0, 0, nr, W, p1[:])
                xx = work.tile([nr, W], FP, tag=f"XX_{r0}")
                x0 = X0t[r0 // 128][2]
                nc.vector.scalar_tensor_tensor(xx[:], s2[:], -8.0, x0[:],
                                               mybir.AluOpType.mult, mybir.AluOpType.add)
                nc.vector.scalar_tensor_tensor(xx[:], p1[:], -SQ2, xx[:],
                                               mybir.AluOpType.mult, mybir.AluOpType.add)
                # strips: p in [4,16): k=-1 rot 256+280; [16,272): k=0; [272,528): k=1; [528,540): k=2
                for (p0s, sr0, cnt, k) in [(4, 244, 12, -1), (16, 0, 256, 0), (272, 0, 256, 1), (528, 0, 12, 2)]:
                    lo = max(sr0, r0)
                    hi = min(sr0 + cnt, r0 + nr)
                    if lo >= hi:
                        continue
                    pp = p0s + (lo - sr0)
                    rot = (256 * k - 280) % W
                    c = 0
                    while c < WET:
                        sc = (c + rot) % W
                        seg = min(WET - c, W - sc)
                        nc.sync.dma_start(dram_ap(EXX, pp * WET + c, [[WET, hi - lo], [1, seg]]),
                                          xx[lo - r0:hi - r0, sc:sc + seg])
                        c += seg

            # ================= stage 7: S3 = conv_f_per(P1Cs, shift11) =================
            # P1Cs_pad[k,c]: (i,j)=(k-6,c-6), A=i+j+1, B=j-i: EXX row A+16, col B+280
            def gather_ij(hd, wid, Aoff, roff, coff, i0, j0, nr, ncol, dst):
                # row = (i+j+Aoff)+roff, col = (j-i)+coff
                p0 = i0 + j0 + Aoff + roff
                q0 = j0 - i0 + coff
                nc.sync.dma_start(dst, dram_ap(hd, p0 * wid + q0,
                                               [[wid - 1, nr], [wid + 1, ncol]]))

            P1Cp = []
            for k0 in range(0, 267, 128):
                kk = min(128, 267 - k0)
                t = work.tile([kk, 267], FP, tag=f"P1Cp_{k0}")
                gather_ij(EXX, WET, 1, 16, 280, k0 - 6, -6, kk, 267, t[:])
                P1Cp.append((k0, kk, t))
            S3t = conv2(P1Cp, 267, 267, "Bf256", "Bf256", HH, HH, keep, "S3")

            # E0s = (P0Bs - 16*S3)/sqrt2
            for (r0, nr, s3) in S3t:
                pb = work.tile([nr, HH], FP, tag=f"P0B_{r0}")
                gather_ij(EXX, WET, 0, 16, 280, r0, 0, nr, HH, pb[:])
                e0 = work.tile([nr, HH], FP, tag=f"E0_{r0}")
                nc.vector.scalar_tensor_tensor(e0[:], s3[:], -16.0, pb[:],
                                               mybir.AluOpType.mult, mybir.AluOpType.add)
                nc.vector.tensor_scalar_mul(e0[:], e0[:], 1.0 / SQ2)
                nc.sync.dma_start(E0d.ap()[r0:r0 + nr, :], e0[:])

            # ================= stage 8: S4 = conv_f_per(E0s, shift00) =================
            E0p = pad_per_from_dram(E0d, HH, HH, 5, 5, 267, 267, work, "E0p")
            S4t = conv2(E0p, 267, 267, "Bf256", "Bf256", HH, HH, keep, "S4")

            # ================= stage 9: output maps =================
            # Amap=-T1e/2-S3/sq2-MEE/2 ; Bmap=T1o-S4+MOO ; Cmap=-T2e/sq2-S3/sq2+MEO ; Emap=sq2*T2o-S4-2*MOE
            Gt = {k: [] for k in ["A", "B", "C", "E"]}
            for ti, (r0, nr, s3) in enumerate(S3t):
                s4 = S4t[ti][2]
                tm = {}
                for nm, hd, Aoff in [("T1e", E1, 0), ("T1o", E1, 1), ("T2e", E2, 0), ("T2o", E2, 1)]:
                    t = work.tile([nr, HH], FP, tag=f"Tm_{nm}")
                    gather_ij(hd, 512, Aoff, 0, 256, r0, 0, nr, HH, t[:])
                    tm[nm] = t
                mp = {}
                for nm, pr, pc in [("MEE", 0, 0), ("MOO", 1, 1), ("MEO", 0, 1), ("MOE", 1, 2)]:
                    t = work.tile([nr, HH], FP, tag=f"Mp_{nm}")
                    if pc < 2:
                        nc.sync.dma_start(t[:], dram_ap(Md, (2 * r0 + pr) * W + pc, [[2 * W, nr], [2, HH]]))
                    else:
                        nc.sync.dma_start(t[:, 0:HH - 1], dram_ap(Md, (2 * r0 + pr) * W + pc, [[2 * W, nr], [2, HH - 1]]))
                        nc.sync.dma_start(t[:, HH - 1:HH], dram_ap(Md, (2 * r0 + pr) * W, [[2 * W, nr], [1, 1]]))
                    mp[nm] = t
                ga = keep.tile([nr, HH], FP, tag=f"GA_{r0}")
                nc.vector.tensor_scalar_mul(ga[:], s3[:], -1.0 / SQ2)
                nc.vector.scalar_tensor_tensor(ga[:], tm["T1e"][:], -0.5, ga[:], mybir.AluOpType.mult, mybir.AluOpType.add)
                nc.vector.scalar_tensor_tensor(ga[:], mp["MEE"][:], -0.5, ga[:], mybir.AluOpType.mult, mybir.AluOpType.add)
                gb = keep.tile([nr, HH], FP, tag=f"GB_{r0}")
                nc.vector.scalar_tensor_tensor(gb[:], s4[:], -1.0, tm["T1o"][:],
                                               mybir.AluOpType.mult, mybir.AluOpType.add)
                nc.vector.scalar_tensor_tensor(gb[:], mp["MOO"][:], 1.0, gb[:], mybir.AluOpType.mult, mybir.AluOpType.add)
                gc = keep.tile([nr, HH], FP, tag=f"GC_{r0}")
                nc.vector.scalar_tensor_tensor(gc[:], tm["T2e"][:], -1.0 / SQ2, mp["MEO"][:],
                                               mybir.AluOpType.mult, mybir.AluOpType.add)
                nc.vector.scalar_tensor_tensor(gc[:], s3[:], -1.0 / SQ2, gc[:], mybir.AluOpType.mult, mybir.AluOpType.add)
                ge = keep.tile([nr, HH], FP, tag=f"GE_{r0}")
                nc.vector.tensor_scalar_mul(ge[:], s4[:], -1.0)
                nc.vector.scalar_tensor_tensor(ge[:], tm["T2o"][:], SQ2, ge[:], mybir.AluOpType.mult, mybir.AluOpType.add)
                nc.vector.scalar_tensor_tensor(ge[:], mp["MOE"][:], -2.0, ge[:], mybir.AluOpType.mult, mybir.AluOpType.add)
                Gt["A"].append((r0, nr, ga)); Gt["B"].append((r0, nr, gb))
                Gt["C"].append((r0, nr, gc)); Gt["E"].append((r0, nr, ge))

            # ================= stage 10: per-channel outputs =================
            # e0lo = xEE/2 + A ; e1lo = -xOO + B ; e0hi = -xEO + C ; e1hi = 2*xOE + E
            specs = [("out_e0lo", 0, 0, 0.5, "A"), ("out_e1lo", 1, 1, -1.0, "B"),
                     ("out_e0hi", 0, 1, -1.0, "C"), ("out_e1hi", 1, 2, 2.0, "E")]
            for ch in range(NCH):
                for (onm, pr, pc, s, gk) in specs:
                    for (r0, nr, g) in Gt[gk]:
                        xt = outp.tile([nr, HH], FP, tag="xg")
                        if pc < 2:
                            nc.sync.dma_start(xt[:], dram_ap(
                                x_h, ch * H * W + (2 * r0 + pr) * W + pc, [[2 * W, nr], [2, HH]]))
                        else:
                            nc.sync.dma_start(xt[:, 0:HH - 1], dram_ap(
                                x_h, ch * H * W + (2 * r0 + pr) * W + pc, [[2 * W, nr], [2, HH - 1]]))
                            nc.sync.dma_start(xt[:, HH - 1:HH], dram_ap(
                                x_h, ch * H * W + (2 * r0 + pr) * W, [[2 * W, nr], [1, 1]]))
                        ot = outp.tile([nr, HH], FP, tag="og")
                        nc.vector.scalar_tensor_tensor(ot[:], xt[:], s, g[:],
                                                 mybir.AluOpType.mult, mybir.AluOpType.add)
                        nc.sync.dma_start(outs[onm].ap()[ch, r0:r0 + nr, :], ot[:])

    nc.compile()
    return nc


def kernel(x, h, g, f):
    import numpy as np
    from concourse import bass_utils
    if "nc" not in _cache:
        _cache["nc"] = _build_nc()
    nc = _cache["nc"]
    hn, gn, fn = _filters()
    mats = _host_mats(np.asarray(h, np.float32), np.asarray(g, np.float32), np.asarray(f, np.float32))
    x = np.ascontiguousarray(np.asarray(x, np.float32))
    in_maps = []
    for i in range(NCORES):
        m = {"x": x[i]}
        m.update(mats)
        in_maps.append(m)
    res = bass_utils.run_bass_kernel_spmd(nc, in_maps, core_ids=list(range(NCORES)))
    def stack(nm):
        return np.stack([res.results[i][nm] for i in range(NCORES)], axis=0)
    return (stack("out_c"), stack("out_e1lo"), stack("out_e0lo"),
            stack("out_e1hi"), stack("out_e0hi"))



# revision 15
# speedup vs baseline: 3.6503x; 3.6503x over previous
import numpy as np

SQ2 = 2.0 ** 0.5
H = W = 512
HH = 256
NCH = 8
NCORES = 8

_cache = {}


def _filters():
    hh = np.array([0.037828455506995, -0.02384946501938, -0.11062440441842, 0.37740285561265], np.float64)
    h = np.concatenate([hh, [0.8526986790094], hh[::-1]])
    gg = np.array([-0.064538882628938, -0.040689417609558, 0.41809227322221], np.float64)
    g = np.concatenate([gg, [0.78848561640566], gg[::-1]])
    v = np.array([0.63, -0.193, 0.0972, -0.0526, 0.0272, -0.0144], np.float64)
    f = np.concatenate([v[::-1], v])
    f[::2] = -f[::2]
    return h, g, f


def _host_mats():
    h, g, f = _filters()
    BhP = np.zeros((520, 256))
    for i in range(256):
        for p in (0, 1):
            r = 2 * i + p
            for u in range(9):
                BhP[r + u, i] += 0.5 * h[u]
    GU = np.zeros((260, 512))
    for r in range(512):
        for u in range(7):
            al = r + u - 3
            if al % 2 == 0:
                GU[al // 2 + 2, r] += g[u]
    Bf256 = np.zeros((267, 256))
    for o in range(256):
        for u in range(12):
            Bf256[o + u, o] = f[u]
    Bf512 = np.zeros((523, 512))
    for o in range(512):
        for u in range(12):
            Bf512[o + u, o] = f[u]
    PI = np.concatenate([np.arange(0, 256, 2), np.arange(256, 512, 2),
                         np.arange(1, 256, 2), np.arange(257, 512, 2)])
    Ah = np.zeros((512, 256))
    for k in range(520):
        Ah[(k - 4) % 512] += BhP[k]
    Ag = np.zeros((256, 512))
    for k in range(260):
        Ag[(k - 2) % 256] += GU[k]
    f32 = np.float32
    return {
        "Ah_r": Ah[PI].astype(f32), "Ah_c": Ah.astype(f32),
        "Ag_r": (8.0 * Ag[:, PI]).astype(f32), "Ag_c": Ag.astype(f32),
        "Bf256": Bf256.astype(f32), "Bf512": Bf512.astype(f32),
        "ident": np.eye(128, dtype=f32),
    }


def _build_nc(debug=False):
    import concourse.bass as bass
    import concourse.bacc as bacc
    import concourse.mybir as mybir
    from concourse import tile

    FP = mybir.dt.float32
    nc = bacc.Bacc("TRN2", target_bir_lowering=False, debug=False, num_devices=NCORES)
    AP = bass.AP
    MUL = mybir.AluOpType.mult
    ADD = mybir.AluOpType.add

    x_h = nc.dram_tensor("x", [NCH, H, W], FP, kind="ExternalInput")
    mat_hs = {}
    for nm, shp in [("Ah_r", (512, 256)), ("Ah_c", (512, 256)), ("Ag_r", (256, 512)),
                    ("Ag_c", (256, 512)), ("Bf256", (267, 256)), ("Bf512", (523, 512)),
                    ("ident", (128, 128))]:
        mat_hs[nm] = nc.dram_tensor(nm, list(shp), FP, kind="ExternalInput")
    outs = {}
    for nm in ["out_c", "out_e1lo", "out_e0lo", "out_e1hi", "out_e0hi"]:
        outs[nm] = nc.dram_tensor(nm, [NCH, HH, HH], FP, kind="ExternalOutput")
    dbg = {}
    if debug:
        for nm, shp in [("dMp", (512, 512)), ("dG0", (128, 1046)), ("dS1n", (256, 512)),
                        ("dT1e", (256, 256)), ("dT1o", (256, 256)), ("dX0", (256, 512)),
                        ("dS2n", (256, 512)), ("dXX", (256, 512)), ("dH0", (128, 534)),
                        ("dS3n", (256, 256)), ("dP0B", (256, 256)), ("dE0", (256, 256)),
                        ("dS4n", (256, 256)), ("dGA", (256, 256)), ("dGB", (256, 256)),
                        ("dGC", (256, 256)), ("dGE", (256, 256))]:
            dbg[nm] = nc.dram_tensor(nm, list(shp), FP, kind="ExternalOutput")

    WD2, ND2 = 790, 789
    D2 = nc.dram_tensor("D2", [ND2, WD2], FP, kind="Internal")
    X0d = nc.dram_tensor("X0d", [HH, W], FP, kind="Internal")
    E1T = nc.dram_tensor("E1T", [W, W], FP, kind="Internal")
    E2T = nc.dram_tensor("E2T", [W, W], FP, kind="Internal")
    WEX, NEX = 538, 547
    EXXT = nc.dram_tensor("EXXT", [NEX, WEX], FP, kind="Internal")
    E0d = nc.dram_tensor("E0d", [HH, HH], FP, kind="Internal")

    def dram_ap(hd, off, dims):
        return AP(hd, off, [list(d) for d in dims])

    with tile.TileContext(nc) as tc:
        import contextlib
        ctx = contextlib.ExitStack()
        with ctx:
            cpool = ctx.enter_context(tc.tile_pool(name="consts", bufs=1))
            keep = ctx.enter_context(tc.tile_pool(name="keep", bufs=1))
            tmp = ctx.enter_context(tc.tile_pool(name="tmp", bufs=1))
            outp = ctx.enter_context(tc.tile_pool(name="outp", bufs=2))
            psum = ctx.enter_context(tc.tile_pool(name="ps", bufs=4, space="PSUM"))

            def load_chunks(nm, rowsets, M, tag):
                tl = []
                for ci, rows in enumerate(rowsets):
                    kk = sum(r1 - r0 for (r0, r1) in rows)
                    t = cpool.tile([kk, M], FP, tag=f"m_{tag}_{ci}")
                    p = 0
                    for (r0, r1) in rows:
                        nc.sync.dma_start(t[p:p + (r1 - r0), :], mat_hs[nm].ap()[r0:r1, :])
                        p += r1 - r0
                    tl.append((kk, t))
                return tl

            nat4 = [[(0, 128)], [(128, 256)], [(256, 384)], [(384, 512)]]
            Ah_r_t = load_chunks("Ah_r", nat4, 256, "ahr")
            Ah_c_t = load_chunks("Ah_c", nat4, 256, "ahc")
            nat2 = [[(0, 128)], [(128, 256)]]
            Ag_r_t = load_chunks("Ag_r", nat2, 512, "agr")
            Ag_c_t = load_chunks("Ag_c", nat2, 512, "agc")
            perm267 = [[(6, 134)], [(134, 262)], [(262, 267), (0, 6)]]
            nat267 = [[(0, 128)], [(128, 256)], [(256, 267)]]
            Bf256p_t = load_chunks("Bf256", perm267, 256, "bfp")
            Bf256n_t = load_chunks("Bf256", nat267, 256, "bfn")
            nat523 = [[(0, 128)], [(128, 256)], [(256, 384)], [(384, 512)], [(512, 523)]]
            Bf512_t = load_chunks("Bf512", nat523, 512, "bf5")
            identt = cpool.tile([128, 128], FP, tag="ident")
            nc.sync.dma_start(identt[:], mat_hs["ident"].ap()[:, :])

            def transpose_tiles(src_tiles, R, C, pool, tag, shared=False):
                # src_tiles: list (r0, nr, tile[nr, C]) covering [R, C] -> tiles of [C, R]
                outt = []
                for c0 in range(0, C, 128):
                    cw = min(128, C - c0)
                    if shared:
                        t = pool.tile([cw, 512], FP, tag=f"{tag}_{c0}")
                    else:
                        t = pool.tile([cw, R], FP, tag=f"{tag}_{c0}")
                    for (r0, nr, st) in src_tiles:
                        ps = psum.tile([cw, nr], FP, tag="ps")
                        nc.tensor.transpose(ps[:, :], st[:, c0:c0 + cw], identt[:nr, :nr])
                        nc.vector.tensor_copy(t[:, r0:r0 + nr], ps[:, :])
                    outt.append((c0, cw, t))
                return outt

            def band_pass(in_specs, mat_tiles, M, N, pool, tag, shared=False):
                # out[m, n] = sum_k mat[k, m] * in[k, n]
                outt = []
                for m0 in range(0, M, 128):
                    mw = min(128, M - m0)
                    if shared:
                        t = pool.tile([mw, 523], FP, tag=f"{tag}_{m0}")
                    else:
                        t = pool.tile([mw, N], FP, tag=f"{tag}_{m0}")
                    for n0 in range(0, N, 512):
                        nw = min(512, N - n0)
                        ps = psum.tile([mw, nw], FP, tag="ps")
                        nk = len(in_specs)
                        for ki, ((kk, fn), (mkk, mt)) in enumerate(zip(in_specs, mat_tiles)):
                            assert kk == mkk
                            nc.tensor.matmul(ps[:, :], mt[:, m0:m0 + mw], fn(n0, nw),
                                             start=(ki == 0), stop=(ki == nk - 1))
                        nc.vector.tensor_copy(t[:, n0:n0 + nw], ps[:, :])
                    outt.append((m0, mw, t))
                return outt

            def specs_of(tiles):
                return [(nr, (lambda t: (lambda n0, nw: t[:, n0:n0 + nw]))(t)) for (_, nr, t) in tiles]

            def conv2v(in_specs, mats_row, mats_col, Mr, Cp):
                # pass1 [Mr<=256, Cp], transpose, pass2 -> [Mc=col-mat-M, Mr] (transposed result)
                p1 = band_pass(in_specs, mats_row, Mr, Cp, tmp, "cvp1", shared=True)
                p1t = transpose_tiles(p1, Mr, Cp, tmp, "cvt1", shared=True)
                Mc = 512 if mats_col is Bf512_t else (512 if len(mats_col) == 2 else 256)
                return band_pass(specs_of(p1t), mats_col, Mc, Mr, tmp, "cvp2", shared=True)

            def pad_per_from_dram(hd, R, C, ru, cl, Rp, Cp, tag, qper=False):
                tiles = []
                for k0 in range(0, Rp, 128):
                    kk = min(128, Rp - k0)
                    t = tmp.tile([kk, 523], FP, tag=f"{tag}_{k0}")
                    k = k0
                    while k < k0 + kk:
                        a = k - ru
                        band = 0 if 0 <= a < R else (-1 if a < 0 else 1)
                        if band == -1:
                            run = min(k0 + kk - k, -a)
                        elif band == 0:
                            run = min(k0 + kk - k, R - a)
                        else:
                            run = k0 + kk - k
                        sr = a % R
                        rot = (C // 2) if (qper and band != 0) else 0
                        c = 0
                        while c < Cp:
                            sc = (c - cl + rot) % C
                            seg = min(Cp - c, C - sc)
                            nc.sync.dma_start(
                                t[k - k0:k - k0 + run, c:c + seg],
                                dram_ap(hd, sr * C + sc, [[C, run], [1, seg]]))
                            c += seg
                        k += run
                    tiles.append((k0, kk, t))
                return tiles

            # ========== stage 1: load x (parity megas) + channel sum ==========
            Xp4 = []
            for j, (par, hhalf) in enumerate([(0, 0), (0, 1), (1, 0), (1, 1)]):
                mg, mgfree = tc.tile([128, NCH * W], FP, name=f"xsum_mega_{j}")
                base = hhalf * 2 * 128 * W + par * W
                nc.sync.dma_start(mg[:], dram_ap(x_h, base, [[2 * W, 128], [H * W, NCH], [1, W]]))
                acc = keep.tile([128, W], FP, tag=f"Xp_{j}")
                nc.vector.scalar_tensor_tensor(acc[:], mg[:, 0:W], 1.0, mg[:, W:2 * W], MUL, ADD)
                for ch in range(2, NCH):
                    nc.vector.scalar_tensor_tensor(acc[:], mg[:, ch * W:(ch + 1) * W], 1.0, acc[:], MUL, ADD)
                mgfree()
                Xp4.append((j * 128, 128, acc))

            # ========== stage 2: cA ==========
            p2 = conv2v(specs_of(Xp4), Ah_r_t, Ah_c_t, 256, 512)
            cAt = transpose_tiles(p2, 256, 256, keep, "cA")
            for (r0, nr, t) in cAt:
                for ch in range(NCH):
                    nc.sync.dma_start(outs["out_c"].ap()[ch, r0:r0 + nr, :], t[:])

            # ========== stage 3: M (rows in PI order) ==========
            m2 = conv2v(specs_of(cAt), Ag_r_t, Ag_c_t, 512, 256)
            Mp = transpose_tiles(m2, 512, 512, keep, "Mp")

            if debug:
                for j in range(4):
                    nc.sync.dma_start(dbg["dMp"].ap()[j * 128:(j + 1) * 128, :], Mp[j][2][:])
            # ========== stage 4: Dsum; write D2 ==========
            colsegs = [(0, 500, 12), (12, 0, 512), (524, 0, 266)]
            for j, off in enumerate([0, 256, 1, 257]):
                d = tmp.tile([128, W], FP, tag=f"Ds_{j % 2}")
                nc.vector.scalar_tensor_tensor(d[:], Mp[j][2][:], -8.0, Xp4[j][2][:], MUL, ADD)
                for (dc, sc, seg) in colsegs:
                    nc.sync.dma_start(
                        dram_ap(D2, (10 + off) * WD2 + dc, [[2 * WD2, 128], [1, seg]]),
                        d[:, sc:sc + seg])
                nb = 128 if off in (0, 1) else (6 if off == 256 else 5)
                for (dc, sc, seg) in colsegs:
                    nc.sync.dma_start(
                        dram_ap(D2, (522 + off) * WD2 + dc, [[2 * WD2, nb], [1, seg]]),
                        d[0:nb, sc:sc + seg])
                # band C: D2 rows 0..9 = Dsum rows 502..511 (u = r - 1024)
                if off in (256, 257):
                    # tile rows p in [123,128) -> r = off + 2p in [502, 511]; D2 row r - 502
                    for (dc, sc, seg) in colsegs:
                        nc.sync.dma_start(
                            dram_ap(D2, (off - 256) * WD2 + dc, [[2 * WD2, 5], [1, seg]]),
                            d[123:128, sc:sc + seg])

            # ========== stage 5: P-pair gathers; S1; E1T; X0; Tp1 ==========
            def ppair_tile(a0, npart, dst):
                nc.sync.dma_start(dst, dram_ap(
                    D2, (528 + a0) * WD2 + 6 + a0, [[WD2 + 1, npart], [-(WD2 - 1), 523], [1, 2]]))

            G0 = keep.tile([128, 1046], FP, tag="G0")
            G1 = keep.tile([128, 1046], FP, tag="G1")
            G2 = keep.tile([11, 1046], FP, tag="G2")
            ppair_tile(0, 128, G0[:])
            ppair_tile(128, 128, G1[:])
            ppair_tile(256, 5, G2[0:5, :])
            ppair_tile(-6, 6, G2[5:11, :])

            if debug:
                nc.sync.dma_start(dbg["dG0"].ap()[:, :], G0[:])

            def pair_specs(gtiles):
                return [(nk, (lambda t: (lambda n0, nw: t[:, 2 * n0 + 1:2 * (n0 + nw):2]))(t))
                        for (nk, t) in gtiles]

            S1T = conv2v(pair_specs([(128, G0), (128, G1), (11, G2)]), Bf256p_t, Bf512_t, 256, 523)
            for (c0, nr, t) in S1T:
                nc.sync.dma_start(dram_ap(E1T, ((c0 + 256) % 512) * W, [[W, 128], [1, 256]]), t[:, 0:256])
                nc.sync.dma_start(dram_ap(E1T, c0 * W + 256, [[W, 128], [1, 256]]), t[:, 0:256])
            S1n = transpose_tiles(S1T, 512, 256, keep, "S1n")
            if debug:
                for (r0, nr, t) in S1n:
                    nc.sync.dma_start(dbg["dS1n"].ap()[r0:r0 + nr, :], t[:, 0:W])
            X0t = []
            for (r0, nr, s1) in S1n:
                g = (G0 if r0 == 0 else G1)
                x0 = keep.tile([128, W], FP, tag=f"X0_{r0}")
                nc.vector.scalar_tensor_tensor(x0[:], s1[:, 0:W], -8.0, g[:, 12:12 + 2 * W:2], MUL, ADD)
                nc.vector.tensor_scalar_mul(x0[:], x0[:], 1.0 / SQ2)
                nc.sync.dma_start(X0d.ap()[r0:r0 + nr, :], x0[:])
                X0t.append((r0, nr, x0))

            # T-pair gather partitioned by b (positive partition stride):
            # TpT[b, 2a+e] = E1T[(b-a+256)*512 + (a+b+e)]
            def tpair(hd, b0, dst):
                nc.sync.dma_start(dst, dram_ap(
                    hd, (b0 + 256) * W + b0, [[W + 1, 128], [-(W - 1), 256], [1, 2]]))

            def tmaps(hd, tagpfx):
                # returns {0: [2 tiles [128a, 256b]], 1: [...]} for e=0 (Te), e=1 (To)
                tpT = []
                for b0 in (0, 128):
                    t = tmp.tile([128, 512], FP, tag=f"tpT_{b0}")
                    tpair(hd, b0, t[:])
                    tpT.append(t)
                res = {}
                for e in (0, 1):
                    cps = []
                    for bi, t in enumerate(tpT):
                        cp = tmp.tile([128, 256], FP, tag=f"tpc_{bi}")
                        nc.vector.tensor_copy(cp[:], t[:, e:512:2])
                        cps.append((bi * 128, 128, cp))
                    res[e] = transpose_tiles(cps, 256, 256, keep, f"{tagpfx}{e}")
                return res

            T1 = tmaps(E1T, "T1_")
            if debug:
                for e, nm in ((0, "dT1e"), (1, "dT1o")):
                    for (c0, cw, t) in T1[e]:
                        nc.sync.dma_start(dbg[nm].ap()[c0:c0 + cw, :], t[:])
                for (r0, nr, x0) in X0t:
                    nc.sync.dma_start(dbg["dX0"].ap()[r0:r0 + nr, :], x0[:])

            # ========== stage 6: S2; E2T; Tp2; XX; EXXT ==========
            X0p = pad_per_from_dram(X0d, HH, W, 5, 5, 267, 523, "padp", qper=True)
            S2T = conv2v(specs_of(X0p), Bf256n_t, Bf512_t, 256, 523)
            for (c0, nr, t) in S2T:
                nc.sync.dma_start(dram_ap(E2T, ((c0 + 256) % 512) * W, [[W, 128], [1, 256]]), t[:, 0:256])
                nc.sync.dma_start(dram_ap(E2T, c0 * W + 256, [[W, 128], [1, 256]]), t[:, 0:256])
            S2n = transpose_tiles(S2T, 512, 256, tmp, "S2n")
            T2 = tmaps(E2T, "T2_")

            XXt = []
            for (r0, nr, s2) in S2n:
                g = (G0 if r0 == 0 else G1)
                xx = tmp.tile([128, W], FP, tag=f"XX_{r0}")
                x0 = X0t[r0 // 128][2]
                nc.vector.scalar_tensor_tensor(xx[:], s2[:, 0:W], -8.0, x0[:], MUL, ADD)
                nc.vector.scalar_tensor_tensor(xx[:], g[:, 13:13 + 2 * W:2], -SQ2, xx[:], MUL, ADD)
                XXt.append((r0, nr, xx))
            XXT = transpose_tiles(XXt, 256, 512, tmp, "XXT")

            if debug:
                for (r0, nr, t) in S2n:
                    nc.sync.dma_start(dbg["dS2n"].ap()[r0:r0 + nr, :], t[:, 0:W])
                for (r0, nr, t) in XXt:
                    nc.sync.dma_start(dbg["dXX"].ap()[r0:r0 + nr, :], t[:])
            for (s0, ns, t) in XXT:
                for (k, pcol0, a0, cnt) in [(0, 16, 0, 256), (1, 272, 0, 256), (-1, 4, 244, 12), (2, 528, 0, 10)]:
                    qb0 = (280 - 256 * k + s0) % 512
                    runs = [(0, qb0, min(ns, 512 - qb0))]
                    if runs[0][2] < ns:
                        runs.append((runs[0][2], 0, ns - runs[0][2]))
                    for (srow, qs, rl) in runs:
                        for qc in (qs, qs + 512):
                            tlo = max(0, 14 - qc)
                            thi = min(rl, NEX - qc)
                            if tlo < thi:
                                nc.sync.dma_start(
                                    dram_ap(EXXT, (qc + tlo) * WEX + pcol0, [[WEX, thi - tlo], [1, cnt]]),
                                    t[srow + tlo:srow + thi, a0:a0 + cnt])

            # ========== stage 7: C-pair gathers (partitioned by j); S3; E0 ==========
            # H[jj, 2*ii+e] = EXXT[(j-i+280)*WEX + (i+j+e+16)], j = j0+p, i = ii-6
            def cpair_tile(j0, npart, dst):
                nc.sync.dma_start(dst, dram_ap(
                    EXXT, (j0 + 286) * WEX + j0 + 10, [[WEX + 1, npart], [-(WEX - 1), 267], [1, 2]]))

            H0 = keep.tile([128, 534], FP, tag="H0")
            H1 = keep.tile([128, 534], FP, tag="H1")
            H2 = keep.tile([11, 534], FP, tag="H2")
            cpair_tile(0, 128, H0[:])
            cpair_tile(128, 128, H1[:])
            cpair_tile(256, 5, H2[0:5, :])
            cpair_tile(-6, 6, H2[5:11, :])

            if debug:
                nc.sync.dma_start(dbg["dH0"].ap()[:, :], H0[:])
            # pass1 contracts j (partition dim of H), pass2 contracts i -> S3 natural
            S3n = conv2v(pair_specs([(128, H0), (128, H1), (11, H2)]), Bf256p_t, Bf256n_t, 256, 267)
            # P0B via strided copy + PE transpose: P0BT[b, a] = H_b[:, 12+2a]
            p0bt = []
            for bi, hgt in enumerate((H0, H1)):
                cp = tmp.tile([128, 256], FP, tag=f"tpc_{bi}")
                nc.vector.tensor_copy(cp[:], hgt[:, 12:12 + 2 * HH:2])
                p0bt.append((bi * 128, 128, cp))
            P0Bn = transpose_tiles(p0bt, 256, 256, tmp, "p0bn")
            if debug:
                for (r0, nr, t) in S3n:
                    nc.sync.dma_start(dbg["dS3n"].ap()[r0:r0 + nr, :], t[:, 0:HH])
                for (c0, cw, t) in P0Bn:
                    nc.sync.dma_start(dbg["dP0B"].ap()[c0:c0 + cw, :], t[:])
            for ((r0, nr, s3), (_, _, p0b)) in zip(S3n, P0Bn):
                e0 = outp.tile([128, HH], FP, tag="E0w")
                nc.vector.scalar_tensor_tensor(e0[:], s3[:, 0:HH], -16.0, p0b[:, 0:HH], MUL, ADD)
                nc.vector.tensor_scalar_mul(e0[:], e0[:], 1.0 / SQ2)
                nc.sync.dma_start(E0d.ap()[r0:r0 + nr, :], e0[:])
                if debug:
                    nc.sync.dma_start(dbg["dE0"].ap()[r0:r0 + nr, :], e0[:])

            # ========== stage 8: S4 ==========
            E0p = pad_per_from_dram(E0d, HH, HH, 5, 5, 267, 267, "padp")
            S4T = conv2v(specs_of(E0p), Bf256n_t, Bf256n_t, 256, 267)
            S4n = transpose_tiles(S4T, 256, 256, keep, "S4n")

            if debug:
                dd = np_none = None
                for (r0, nr, t) in S4n:
                    nc.sync.dma_start(dbg["dS4n"].ap()[r0:r0 + nr, :], t[:, 0:HH])
            # ========== stage 9: broadcast maps ==========
            Gmaps = {k: [] for k in "ABCE"}
            for ti in range(2):
                r0 = ti * 128
                s3 = S3n[ti][2]
                s4 = S4n[ti][2]
                t1e = T1[0][ti][2]
                t1o = T1[1][ti][2]
                t2e = T2[0][ti][2]
                t2o = T2[1][ti][2]
                me = Mp[ti][2]
                mo = Mp[2 + ti][2]
                ga = keep.tile([128, HH], FP, tag=f"GA_{r0}")
                nc.vector.tensor_scalar_mul(ga[:], s3[:, 0:HH], -1.0 / SQ2)
                nc.vector.scalar_tensor_tensor(ga[:], t1e[:, 0:HH], -0.5, ga[:], MUL, ADD)
                nc.vector.scalar_tensor_tensor(ga[:], me[:, 0:W:2], -0.5, ga[:], MUL, ADD)
                gb = keep.tile([128, HH], FP, tag=f"GB_{r0}")
                nc.vector.scalar_tensor_tensor(gb[:], s4[:, 0:HH], -1.0, t1o[:, 0:HH], MUL, ADD)
                nc.vector.scalar_tensor_tensor(gb[:], mo[:, 1:W:2], 1.0, gb[:], MUL, ADD)
                gc = keep.tile([128, HH], FP, tag=f"GC_{r0}")
                nc.vector.tensor_scalar_mul(gc[:], s3[:, 0:HH], -1.0 / SQ2)
                nc.vector.scalar_tensor_tensor(gc[:], t2e[:, 0:HH], -1.0 / SQ2, gc[:], MUL, ADD)
                nc.vector.scalar_tensor_tensor(gc[:], me[:, 1:W:2], 1.0, gc[:], MUL, ADD)
                ge = keep.tile([128, HH], FP, tag=f"GE_{r0}")
                nc.vector.tensor_scalar_mul(ge[:], s4[:, 0:HH], -1.0)
                nc.vector.scalar_tensor_tensor(ge[:], t2o[:, 0:HH], SQ2, ge[:], MUL, ADD)
                nc.vector.scalar_tensor_tensor(ge[:, 0:HH - 1], mo[:, 2:W:2], -2.0, ge[:, 0:HH - 1], MUL, ADD)
                nc.vector.scalar_tensor_tensor(ge[:, HH - 1:HH], mo[:, 0:1], -2.0, ge[:, HH - 1:HH], MUL, ADD)
                Gmaps["A"].append(ga)
                Gmaps["B"].append(gb)
                Gmaps["C"].append(gc)
                Gmaps["E"].append(ge)

            if debug:
                for ti, r0 in ((0, 0), (1, 128)):
                    for gk, nm in (("A", "dGA"), ("B", "dGB"), ("C", "dGC"), ("E", "dGE")):
                        nc.sync.dma_start(dbg[nm].ap()[r0:r0 + 128, :], Gmaps[gk][ti][:])
            # ========== stage 10: per-channel outputs ==========
            for hhalf in range(2):
                xe, xefree = tc.tile([128, NCH * W], FP, name=f"xout_e_{hhalf}")
                xo, xofree = tc.tile([128, NCH * W], FP, name=f"xout_o_{hhalf}")
                base = hhalf * 2 * 128 * W
                nc.sync.dma_start(xe[:], dram_ap(x_h, base, [[2 * W, 128], [H * W, NCH], [1, W]]))
                nc.sync.dma_start(xo[:], dram_ap(x_h, base + W, [[2 * W, 128], [H * W, NCH], [1, W]]))
                ga, gb, gc, ge = (Gmaps[k][hhalf] for k in "ABCE")
                r0 = hhalf * 128
                for ch in range(NCH):
                    co = ch * W
                    ot = outp.tile([128, HH], FP, tag="o_e0lo")
                    nc.vector.scalar_tensor_tensor(ot[:], xe[:, co:co + W:2], 0.5, ga[:], MUL, ADD)
                    nc.sync.dma_start(outs["out_e0lo"].ap()[ch, r0:r0 + 128, :], ot[:])
                    ot = outp.tile([128, HH], FP, tag="o_e1lo")
                    nc.vector.scalar_tensor_tensor(ot[:], xo[:, co + 1:co + W:2], -1.0, gb[:], MUL, ADD)
                    nc.sync.dma_start(outs["out_e1lo"].ap()[ch, r0:r0 + 128, :], ot[:])
                    ot = outp.tile([128, HH], FP, tag="o_e0hi")
                    nc.vector.scalar_tensor_tensor(ot[:], xe[:, co + 1:co + W:2], -1.0, gc[:], MUL, ADD)
                    nc.sync.dma_start(outs["out_e0hi"].ap()[ch, r0:r0 + 128, :], ot[:])
                    ot = outp.tile([128, HH], FP, tag="o_e1hi")
                    nc.vector.scalar_tensor_tensor(ot[:, 0:HH - 1], xo[:, co + 2:co + W:2], 2.0, ge[:, 0:HH - 1], MUL, ADD)
                    nc.vector.scalar_tensor_tensor(ot[:, HH - 1:HH], xo[:, co:co + 1], 2.0, ge[:, HH - 1:HH], MUL, ADD)
                    nc.sync.dma_start(outs["out_e1hi"].ap()[ch, r0:r0 + 128, :], ot[:])
                xofree()
                xefree()

    nc.compile()
    return nc


def kernel(x, h, g, f):
    import numpy as np
    from concourse import bass_utils
    if "nc" not in _cache:
        _cache["nc"] = _build_nc()
        _cache["mats"] = _host_mats()
    nc = _cache["nc"]
    mats = _cache["mats"]
    x = np.ascontiguousarray(np.asarray(x, np.float32))
    in_maps = []
    for i in range(NCORES):
        m = {"x": x[i]}
        m.update(mats)
        in_maps.append(m)
    res = bass_utils.run_bass_kernel_spmd(nc, in_maps, core_ids=list(range(NCORES)))

    def stack(nm):
        return np.stack([res.results[i][nm] for i in range(NCORES)], axis=0)

    return (stack("out_c"), stack("out_e1lo"), stack("out_e0lo"),
            stack("out_e1hi"), stack("out_e0hi"))


# revision 17
# speedup vs baseline: 3.6999x; 1.0136x over previous
import numpy as np

SQ2 = 2.0 ** 0.5
H = W = 512
HH = 256
NCH = 8
NCORES = 8

_cache = {}


def _filters():
    hh = np.array([0.037828455506995, -0.02384946501938, -0.11062440441842, 0.37740285561265], np.float64)
    h = np.concatenate([hh, [0.8526986790094], hh[::-1]])
    gg = np.array([-0.064538882628938, -0.040689417609558, 0.41809227322221], np.float64)
    g = np.concatenate([gg, [0.78848561640566], gg[::-1]])
    v = np.array([0.63, -0.193, 0.0972, -0.0526, 0.0272, -0.0144], np.float64)
    f = np.concatenate([v[::-1], v])
    f[::2] = -f[::2]
    return h, g, f


def _host_mats():
    h, g, f = _filters()
    BhP = np.zeros((520, 256))
    for i in range(256):
        for p in (0, 1):
            r = 2 * i + p
            for u in range(9):
                BhP[r + u, i] += 0.5 * h[u]
    GU = np.zeros((260, 512))
    for r in range(512):
        for u in range(7):
            al = r + u - 3
            if al % 2 == 0:
                GU[al // 2 + 2, r] += g[u]
    Bf256 = np.zeros((267, 256))
    for o in range(256):
        for u in range(12):
            Bf256[o + u, o] = f[u]
    Bf512 = np.zeros((523, 512))
    for o in range(512):
        for u in range(12):
            Bf512[o + u, o] = f[u]
    PI = np.concatenate([np.arange(0, 256, 2), np.arange(256, 512, 2),
                         np.arange(1, 256, 2), np.arange(257, 512, 2)])
    Ah = np.zeros((512, 256))
    for k in range(520):
        Ah[(k - 4) % 512] += BhP[k]
    Ag = np.zeros((256, 512))
    for k in range(260):
        Ag[(k - 2) % 256] += GU[k]
    f32 = np.float32
    return {
        "Ah_r": Ah[PI].astype(f32), "Ah_c": Ah.astype(f32),
        "Ag_r": (8.0 * Ag[:, PI]).astype(f32), "Ag_c": Ag.astype(f32),
        "Bf256": Bf256.astype(f32), "Bf512": Bf512.astype(f32),
        "ident": np.eye(128, dtype=f32),
    }


def _build_nc(debug=False):
    import concourse.bass as bass
    import concourse.bacc as bacc
    import concourse.mybir as mybir
    from concourse import tile

    FP = mybir.dt.float32
    nc = bacc.Bacc("TRN2", target_bir_lowering=False, debug=False, num_devices=NCORES)
    AP = bass.AP
    MUL = mybir.AluOpType.mult
    ADD = mybir.AluOpType.add

    x_h = nc.dram_tensor("x", [NCH, H, W], FP, kind="ExternalInput")
    mat_hs = {}
    for nm, shp in [("Ah_r", (512, 256)), ("Ah_c", (512, 256)), ("Ag_r", (256, 512)),
                    ("Ag_c", (256, 512)), ("Bf256", (267, 256)), ("Bf512", (523, 512)),
                    ("ident", (128, 128))]:
        mat_hs[nm] = nc.dram_tensor(nm, list(shp), FP, kind="ExternalInput")
    outs = {}
    for nm in ["out_c", "out_e1lo", "out_e0lo", "out_e1hi", "out_e0hi"]:
        outs[nm] = nc.dram_tensor(nm, [NCH, HH, HH], FP, kind="ExternalOutput")
    dbg = {}
    if debug:
        for nm, shp in [("dMp", (512, 512)), ("dG0", (128, 1046)), ("dS1n", (256, 512)),
                        ("dT1e", (256, 256)), ("dT1o", (256, 256)), ("dX0", (256, 512)),
                        ("dS2n", (256, 512)), ("dXX", (256, 512)), ("dH0", (128, 534)),
                        ("dS3n", (256, 256)), ("dP0B", (256, 256)), ("dE0", (256, 256)),
                        ("dS4n", (256, 256)), ("dGA", (256, 256)), ("dGB", (256, 256)),
                        ("dGC", (256, 256)), ("dGE", (256, 256))]:
            dbg[nm] = nc.dram_tensor(nm, list(shp), FP, kind="ExternalOutput")

    WD2, ND2 = 790, 789
    D2 = nc.dram_tensor("D2", [ND2, WD2], FP, kind="Internal")
    X0d = nc.dram_tensor("X0d", [HH, W], FP, kind="Internal")
    E1T = nc.dram_tensor("E1T", [W, W], FP, kind="Internal")
    E2T = nc.dram_tensor("E2T", [W, W], FP, kind="Internal")
    WEX, NEX = 538, 547
    EXXT = nc.dram_tensor("EXXT", [NEX, WEX], FP, kind="Internal")
    E0d = nc.dram_tensor("E0d", [HH, HH], FP, kind="Internal")

    def dram_ap(hd, off, dims):
        return AP(hd, off, [list(d) for d in dims])

    with tile.TileContext(nc) as tc:
        import contextlib
        ctx = contextlib.ExitStack()
        with ctx:
            cpool = ctx.enter_context(tc.tile_pool(name="consts", bufs=1))
            keep = ctx.enter_context(tc.tile_pool(name="keep", bufs=1))
            tmp = ctx.enter_context(tc.tile_pool(name="tmp", bufs=1))
            outp = ctx.enter_context(tc.tile_pool(name="outp", bufs=2))
            psum = ctx.enter_context(tc.tile_pool(name="ps", bufs=4, space="PSUM"))

            def load_chunks(nm, rowsets, M, tag):
                tl = []
                for ci, rows in enumerate(rowsets):
                    kk = sum(r1 - r0 for (r0, r1) in rows)
                    t = cpool.tile([kk, M], FP, tag=f"m_{tag}_{ci}")
                    p = 0
                    for (r0, r1) in rows:
                        nc.sync.dma_start(t[p:p + (r1 - r0), :], mat_hs[nm].ap()[r0:r1, :])
                        p += r1 - r0
                    tl.append((kk, t))
                return tl

            nat4 = [[(0, 128)], [(128, 256)], [(256, 384)], [(384, 512)]]
            Ah_r_t = load_chunks("Ah_r", nat4, 256, "ahr")
            Ah_c_t = load_chunks("Ah_c", nat4, 256, "ahc")
            nat2 = [[(0, 128)], [(128, 256)]]
            Ag_r_t = load_chunks("Ag_r", nat2, 512, "agr")
            Ag_c_t = load_chunks("Ag_c", nat2, 512, "agc")
            perm267 = [[(6, 134)], [(134, 262)], [(262, 267), (0, 6)]]
            nat267 = [[(0, 128)], [(128, 256)], [(256, 267)]]
            Bf256p_t = load_chunks("Bf256", perm267, 256, "bfp")
            Bf256n_t = load_chunks("Bf256", nat267, 256, "bfn")
            nat523 = [[(0, 128)], [(128, 256)], [(256, 384)], [(384, 512)], [(512, 523)]]
            Bf512_t = load_chunks("Bf512", nat523, 512, "bf5")
            identt = cpool.tile([128, 128], FP, tag="ident")
            nc.sync.dma_start(identt[:], mat_hs["ident"].ap()[:, :])

            _cpctr = [0]

            def psum_copy(dst, src):
                _cpctr[0] += 1
                if _cpctr[0] % 2 == 0:
                    nc.scalar.activation(dst, src, mybir.ActivationFunctionType.Copy)
                else:
                    nc.vector.tensor_copy(dst, src)

            def transpose_tiles(src_tiles, R, C, pool, tag, shared=False):
                # src_tiles: list (r0, nr, tile[nr, C]) covering [R, C] -> tiles of [C, R]
                outt = []
                for c0 in range(0, C, 128):
                    cw = min(128, C - c0)
                    if shared:
                        t = pool.tile([cw, 512], FP, tag=f"{tag}_{c0}")
                    else:
                        t = pool.tile([cw, R], FP, tag=f"{tag}_{c0}")
                    for (r0, nr, st) in src_tiles:
                        ps = psum.tile([cw, nr], FP, tag="ps")
                        nc.tensor.transpose(ps[:, :], st[:, c0:c0 + cw], identt[:nr, :nr])
                        psum_copy(t[:, r0:r0 + nr], ps[:, :])
                    outt.append((c0, cw, t))
                return outt

            def band_pass(in_specs, mat_tiles, M, N, pool, tag, shared=False):
                # out[m, n] = sum_k mat[k, m] * in[k, n]
                outt = []
                for m0 in range(0, M, 128):
                    mw = min(128, M - m0)
                    if shared:
                        t = pool.tile([mw, 523], FP, tag=f"{tag}_{m0}")
                    else:
                        t = pool.tile([mw, N], FP, tag=f"{tag}_{m0}")
                    for n0 in range(0, N, 512):
                        nw = min(512, N - n0)
                        ps = psum.tile([mw, nw], FP, tag="ps")
                        nk = len(in_specs)
                        for ki, ((kk, fn), (mkk, mt)) in enumerate(zip(in_specs, mat_tiles)):
                            assert kk == mkk
                            nc.tensor.matmul(ps[:, :], mt[:, m0:m0 + mw], fn(n0, nw),
                                             start=(ki == 0), stop=(ki == nk - 1))
                        psum_copy(t[:, n0:n0 + nw], ps[:, :])
                    outt.append((m0, mw, t))
                return outt

            def specs_of(tiles):
                return [(nr, (lambda t: (lambda n0, nw: t[:, n0:n0 + nw]))(t)) for (_, nr, t) in tiles]

            def conv2v(in_specs, mats_row, mats_col, Mr, Cp):
                # pass1 [Mr<=256, Cp], transpose, pass2 -> [Mc=col-mat-M, Mr] (transposed result)
                p1 = band_pass(in_specs, mats_row, Mr, Cp, tmp, "cvp1", shared=True)
                p1t = transpose_tiles(p1, Mr, Cp, tmp, "cvt1", shared=True)
                Mc = 512 if mats_col is Bf512_t else (512 if len(mats_col) == 2 else 256)
                return band_pass(specs_of(p1t), mats_col, Mc, Mr, tmp, "cvp2", shared=True)

            def pad_per_from_dram(hd, R, C, ru, cl, Rp, Cp, tag, qper=False):
                tiles = []
                for k0 in range(0, Rp, 128):
                    kk = min(128, Rp - k0)
                    t = tmp.tile([kk, 523], FP, tag=f"{tag}_{k0}")
                    k = k0
                    while k < k0 + kk:
                        a = k - ru
                        band = 0 if 0 <= a < R else (-1 if a < 0 else 1)
                        if band == -1:
                            run = min(k0 + kk - k, -a)
                        elif band == 0:
                            run = min(k0 + kk - k, R - a)
                        else:
                            run = k0 + kk - k
                        sr = a % R
                        rot = (C // 2) if (qper and band != 0) else 0
                        c = 0
                        while c < Cp:
                            sc = (c - cl + rot) % C
                            seg = min(Cp - c, C - sc)
                            nc.sync.dma_start(
                                t[k - k0:k - k0 + run, c:c + seg],
                                dram_ap(hd, sr * C + sc, [[C, run], [1, seg]]))
                            c += seg
                        k += run
                    tiles.append((k0, kk, t))
                return tiles

            # ========== stage 1: load x (parity megas) + channel sum ==========
            Xp4 = []
            for j, (par, hhalf) in enumerate([(0, 0), (0, 1), (1, 0), (1, 1)]):
                mg, mgfree = tc.tile([128, NCH * W], FP, name=f"xsum_mega_{j}")
                base = hhalf * 2 * 128 * W + par * W
                nc.sync.dma_start(mg[:], dram_ap(x_h, base, [[2 * W, 128], [H * W, NCH], [1, W]]))
                acc = keep.tile([128, W], FP, tag=f"Xp_{j}")
                nc.vector.scalar_tensor_tensor(acc[:], mg[:, 0:W], 1.0, mg[:, W:2 * W], MUL, ADD)
                for ch in range(2, NCH):
                    nc.vector.scalar_tensor_tensor(acc[:], mg[:, ch * W:(ch + 1) * W], 1.0, acc[:], MUL, ADD)
                mgfree()
                Xp4.append((j * 128, 128, acc))

            # ========== stage 2: cA ==========
            p2 = conv2v(specs_of(Xp4), Ah_r_t, Ah_c_t, 256, 512)
            cAt = transpose_tiles(p2, 256, 256, keep, "cA")
            for (r0, nr, t) in cAt:
                for ch in range(NCH):
                    nc.sync.dma_start(outs["out_c"].ap()[ch, r0:r0 + nr, :], t[:])

            # ========== stage 3: M (rows in PI order) ==========
            m2 = conv2v(specs_of(cAt), Ag_r_t, Ag_c_t, 512, 256)
            Mp = transpose_tiles(m2, 512, 512, keep, "Mp")

            if debug:
                for j in range(4):
                    nc.sync.dma_start(dbg["dMp"].ap()[j * 128:(j + 1) * 128, :], Mp[j][2][:])
            # ========== stage 4: Dsum; write D2 ==========
            colsegs = [(0, 500, 12), (12, 0, 512), (524, 0, 266)]
            for j, off in enumerate([0, 256, 1, 257]):
                d = tmp.tile([128, W], FP, tag=f"Ds_{j % 2}")
                nc.vector.scalar_tensor_tensor(d[:], Mp[j][2][:], -8.0, Xp4[j][2][:], MUL, ADD)
                for (dc, sc, seg) in colsegs:
                    nc.sync.dma_start(
                        dram_ap(D2, (10 + off) * WD2 + dc, [[2 * WD2, 128], [1, seg]]),
                        d[:, sc:sc + seg])
                nb = 128 if off in (0, 1) else (6 if off == 256 else 5)
                for (dc, sc, seg) in colsegs:
                    nc.sync.dma_start(
                        dram_ap(D2, (522 + off) * WD2 + dc, [[2 * WD2, nb], [1, seg]]),
                        d[0:nb, sc:sc + seg])
                # band C: D2 rows 0..9 = Dsum rows 502..511 (u = r - 1024)
                if off in (256, 257):
                    # tile rows p in [123,128) -> r = off + 2p in [502, 511]; D2 row r - 502
                    for (dc, sc, seg) in colsegs:
                        nc.sync.dma_start(
                            dram_ap(D2, (off - 256) * WD2 + dc, [[2 * WD2, 5], [1, seg]]),
                            d[123:128, sc:sc + seg])

            # early stage-10 x loads for half 0 (fills DMA idle during conv phases)
            xe0, xe0free = tc.tile([128, NCH * W], FP, name="xout_e_0")
            xo0, xo0free = tc.tile([128, NCH * W], FP, name="xout_o_0")
            nc.sync.dma_start(xe0[:], dram_ap(x_h, 0, [[2 * W, 128], [H * W, NCH], [1, W]]))
            nc.sync.dma_start(xo0[:], dram_ap(x_h, W, [[2 * W, 128], [H * W, NCH], [1, W]]))

            # ========== stage 5: P-pair gathers; S1; E1T; X0; Tp1 ==========
            def ppair_tile(a0, npart, dst):
                nc.sync.dma_start(dst, dram_ap(
                    D2, (528 + a0) * WD2 + 6 + a0, [[WD2 + 1, npart], [-(WD2 - 1), 523], [1, 2]]))

            G0 = keep.tile([128, 1046], FP, tag="G0")
            G1 = keep.tile([128, 1046], FP, tag="G1")
            G2 = keep.tile([11, 1046], FP, tag="G2")
            ppair_tile(0, 128, G0[:])
            ppair_tile(128, 128, G1[:])
            ppair_tile(256, 5, G2[0:5, :])
            ppair_tile(-6, 6, G2[5:11, :])

            if debug:
                nc.sync.dma_start(dbg["dG0"].ap()[:, :], G0[:])

            def pair_specs(gtiles):
                return [(nk, (lambda t: (lambda n0, nw: t[:, 2 * n0 + 1:2 * (n0 + nw):2]))(t))
                        for (nk, t) in gtiles]

            S1T = conv2v(pair_specs([(128, G0), (128, G1), (11, G2)]), Bf256p_t, Bf512_t, 256, 523)
            for (c0, nr, t) in S1T:
                nc.sync.dma_start(dram_ap(E1T, ((c0 + 256) % 512) * W, [[W, 128], [1, 256]]), t[:, 0:256])
                nc.sync.dma_start(dram_ap(E1T, c0 * W + 256, [[W, 128], [1, 256]]), t[:, 0:256])
            S1n = transpose_tiles(S1T, 512, 256, keep, "S1n")
            if debug:
                for (r0, nr, t) in S1n:
                    nc.sync.dma_start(dbg["dS1n"].ap()[r0:r0 + nr, :], t[:, 0:W])
            X0t = []
            for (r0, nr, s1) in S1n:
                g = (G0 if r0 == 0 else G1)
                x0 = keep.tile([128, W], FP, tag=f"X0_{r0}")
                nc.vector.scalar_tensor_tensor(x0[:], s1[:, 0:W], -8.0, g[:, 12:12 + 2 * W:2], MUL, ADD)
                nc.vector.tensor_scalar_mul(x0[:], x0[:], 1.0 / SQ2)
                nc.sync.dma_start(X0d.ap()[r0:r0 + nr, :], x0[:])
                X0t.append((r0, nr, x0))

            # T-pair gather partitioned by b (positive partition stride):
            # TpT[b, 2a+e] = E1T[(b-a+256)*512 + (a+b+e)]
            def tpair(hd, b0, dst):
                nc.sync.dma_start(dst, dram_ap(
                    hd, (b0 + 256) * W + b0, [[W + 1, 128], [-(W - 1), 256], [1, 2]]))

            def tmaps(hd, tagpfx):
                # returns {0: [2 tiles [128a, 256b]], 1: [...]} for e=0 (Te), e=1 (To)
                tpT = []
                for b0 in (0, 128):
                    t = tmp.tile([128, 512], FP, tag=f"tpT_{b0}")
                    tpair(hd, b0, t[:])
                    tpT.append(t)
                res = {}
                for e in (0, 1):
                    cps = []
                    for bi, t in enumerate(tpT):
                        cp = tmp.tile([128, 256], FP, tag=f"tpc_{bi}")
                        nc.vector.tensor_copy(cp[:], t[:, e:512:2])
                        cps.append((bi * 128, 128, cp))
                    res[e] = transpose_tiles(cps, 256, 256, keep, f"{tagpfx}{e}")
                return res

            T1 = tmaps(E1T, "T1_")
            if debug:
                for e, nm in ((0, "dT1e"), (1, "dT1o")):
                    for (c0, cw, t) in T1[e]:
                        nc.sync.dma_start(dbg[nm].ap()[c0:c0 + cw, :], t[:])
                for (r0, nr, x0) in X0t:
                    nc.sync.dma_start(dbg["dX0"].ap()[r0:r0 + nr, :], x0[:])

            # ========== stage 6: S2; E2T; Tp2; XX; EXXT ==========
            X0p = pad_per_from_dram(X0d, HH, W, 5, 5, 267, 523, "padp", qper=True)
            S2T = conv2v(specs_of(X0p), Bf256n_t, Bf512_t, 256, 523)
            for (c0, nr, t) in S2T:
                nc.sync.dma_start(dram_ap(E2T, ((c0 + 256) % 512) * W, [[W, 128], [1, 256]]), t[:, 0:256])
                nc.sync.dma_start(dram_ap(E2T, c0 * W + 256, [[W, 128], [1, 256]]), t[:, 0:256])
            S2n = transpose_tiles(S2T, 512, 256, tmp, "S2n")
            T2 = tmaps(E2T, "T2_")

            XXt = []
            for (r0, nr, s2) in S2n:
                g = (G0 if r0 == 0 else G1)
                xx = tmp.tile([128, W], FP, tag=f"XX_{r0}")
                x0 = X0t[r0 // 128][2]
                nc.vector.scalar_tensor_tensor(xx[:], s2[:, 0:W], -8.0, x0[:], MUL, ADD)
                nc.vector.scalar_tensor_tensor(xx[:], g[:, 13:13 + 2 * W:2], -SQ2, xx[:], MUL, ADD)
                XXt.append((r0, nr, xx))
            XXT = transpose_tiles(XXt, 256, 512, tmp, "XXT")

            if debug:
                for (r0, nr, t) in S2n:
                    nc.sync.dma_start(dbg["dS2n"].ap()[r0:r0 + nr, :], t[:, 0:W])
                for (r0, nr, t) in XXt:
                    nc.sync.dma_start(dbg["dXX"].ap()[r0:r0 + nr, :], t[:])
            for (s0, ns, t) in XXT:
                for (k, pcol0, a0, cnt) in [(0, 16, 0, 256), (1, 272, 0, 256), (-1, 4, 244, 12), (2, 528, 0, 10)]:
                    qb0 = (280 - 256 * k + s0) % 512
                    runs = [(0, qb0, min(ns, 512 - qb0))]
                    if runs[0][2] < ns:
                        runs.append((runs[0][2], 0, ns - runs[0][2]))
                    for (srow, qs, rl) in runs:
                        for qc in (qs, qs + 512):
                            tlo = max(0, 14 - qc)
                            thi = min(rl, NEX - qc)
                            if tlo < thi:
                                nc.sync.dma_start(
                                    dram_ap(EXXT, (qc + tlo) * WEX + pcol0, [[WEX, thi - tlo], [1, cnt]]),
                                    t[srow + tlo:srow + thi, a0:a0 + cnt])

            # ========== stage 7: C-pair gathers (partitioned by j); S3; E0 ==========
            # H[jj, 2*ii+e] = EXXT[(j-i+280)*WEX + (i+j+e+16)], j = j0+p, i = ii-6
            def cpair_tile(j0, npart, dst):
                nc.sync.dma_start(dst, dram_ap(
                    EXXT, (j0 + 286) * WEX + j0 + 10, [[WEX + 1, npart], [-(WEX - 1), 267], [1, 2]]))

            H0 = keep.tile([128, 534], FP, tag="H0")
            H1 = keep.tile([128, 534], FP, tag="H1")
            H2 = keep.tile([11, 534], FP, tag="H2")
            cpair_tile(0, 128, H0[:])
            cpair_tile(128, 128, H1[:])
            cpair_tile(256, 5, H2[0:5, :])
            cpair_tile(-6, 6, H2[5:11, :])

            if debug:
                nc.sync.dma_start(dbg["dH0"].ap()[:, :], H0[:])
            # pass1 contracts j (partition dim of H), pass2 contracts i -> S3 natural
            S3n = conv2v(pair_specs([(128, H0), (128, H1), (11, H2)]), Bf256p_t, Bf256n_t, 256, 267)
            # P0B via strided copy + PE transpose: P0BT[b, a] = H_b[:, 12+2a]
            p0bt = []
            for bi, hgt in enumerate((H0, H1)):
                cp = tmp.tile([128, 256], FP, tag=f"tpc_{bi}")
                nc.vector.tensor_copy(cp[:], hgt[:, 12:12 + 2 * HH:2])
                p0bt.append((bi * 128, 128, cp))
            P0Bn = transpose_tiles(p0bt, 256, 256, tmp, "p0bn")
            if debug:
                for (r0, nr, t) in S3n:
                    nc.sync.dma_start(dbg["dS3n"].ap()[r0:r0 + nr, :], t[:, 0:HH])
                for (c0, cw, t) in P0Bn:
                    nc.sync.dma_start(dbg["dP0B"].ap()[c0:c0 + cw, :], t[:])
            for ((r0, nr, s3), (_, _, p0b)) in zip(S3n, P0Bn):
                e0 = outp.tile([128, HH], FP, tag="E0w")
                nc.vector.scalar_tensor_tensor(e0[:], s3[:, 0:HH], -16.0, p0b[:, 0:HH], MUL, ADD)
                nc.vector.tensor_scalar_mul(e0[:], e0[:], 1.0 / SQ2)
                nc.sync.dma_start(E0d.ap()[r0:r0 + nr, :], e0[:])
                if debug:
                    nc.sync.dma_start(dbg["dE0"].ap()[r0:r0 + nr, :], e0[:])

            # ========== stage 8: S4 ==========
            E0p = pad_per_from_dram(E0d, HH, HH, 5, 5, 267, 267, "padp")
            S4T = conv2v(specs_of(E0p), Bf256n_t, Bf256n_t, 256, 267)
            S4n = transpose_tiles(S4T, 256, 256, keep, "S4n")

            if debug:
                dd = np_none = None
                for (r0, nr, t) in S4n:
                    nc.sync.dma_start(dbg["dS4n"].ap()[r0:r0 + nr, :], t[:, 0:HH])
            # ========== stage 9: broadcast maps ==========
            Gmaps = {k: [] for k in "ABCE"}
            for ti in range(2):
                r0 = ti * 128
                s3 = S3n[ti][2]
                s4 = S4n[ti][2]
                t1e = T1[0][ti][2]
                t1o = T1[1][ti][2]
                t2e = T2[0][ti][2]
                t2o = T2[1][ti][2]
                me = Mp[ti][2]
                mo = Mp[2 + ti][2]
                ga = keep.tile([128, HH], FP, tag=f"GA_{r0}")
                nc.vector.tensor_scalar_mul(ga[:], s3[:, 0:HH], -1.0 / SQ2)
                nc.vector.scalar_tensor_tensor(ga[:], t1e[:, 0:HH], -0.5, ga[:], MUL, ADD)
                nc.vector.scalar_tensor_tensor(ga[:], me[:, 0:W:2], -0.5, ga[:], MUL, ADD)
                gb = keep.tile([128, HH], FP, tag=f"GB_{r0}")
                nc.vector.scalar_tensor_tensor(gb[:], s4[:, 0:HH], -1.0, t1o[:, 0:HH], MUL, ADD)
                nc.vector.scalar_tensor_tensor(gb[:], mo[:, 1:W:2], 1.0, gb[:], MUL, ADD)
                gc = keep.tile([128, HH], FP, tag=f"GC_{r0}")
                nc.vector.tensor_scalar_mul(gc[:], s3[:, 0:HH], -1.0 / SQ2)
                nc.vector.scalar_tensor_tensor(gc[:], t2e[:, 0:HH], -1.0 / SQ2, gc[:], MUL, ADD)
                nc.vector.scalar_tensor_tensor(gc[:], me[:, 1:W:2], 1.0, gc[:], MUL, ADD)
                ge = keep.tile([128, HH], FP, tag=f"GE_{r0}")
                nc.vector.tensor_scalar_mul(ge[:], s4[:, 0:HH], -1.0)
                nc.vector.scalar_tensor_tensor(ge[:], t2o[:, 0:HH], SQ2, ge[:], MUL, ADD)
                nc.vector.scalar_tensor_tensor(ge[:, 0:HH - 1], mo[:, 2:W:2], -2.0, ge[:, 0:HH - 1], MUL, ADD)
                nc.vector.scalar_tensor_tensor(ge[:, HH - 1:HH], mo[:, 0:1], -2.0, ge[:, HH - 1:HH], MUL, ADD)
                Gmaps["A"].append(ga)
                Gmaps["B"].append(gb)
                Gmaps["C"].append(gc)
                Gmaps["E"].append(ge)

            if debug:
                for ti, r0 in ((0, 0), (1, 128)):
                    for gk, nm in (("A", "dGA"), ("B", "dGB"), ("C", "dGC"), ("E", "dGE")):
                        nc.sync.dma_start(dbg[nm].ap()[r0:r0 + 128, :], Gmaps[gk][ti][:])
            # ========== stage 10: per-channel outputs ==========
            for hhalf in range(2):
                if hhalf == 0:
                    xe, xo = xe0, xo0
                    xefree, xofree = xe0free, xo0free
                else:
                    xe, xefree = tc.tile([128, NCH * W], FP, name="xout_e_1")
                    xo, xofree = tc.tile([128, NCH * W], FP, name="xout_o_1")
                    base = hhalf * 2 * 128 * W
                    nc.sync.dma_start(xe[:], dram_ap(x_h, base, [[2 * W, 128], [H * W, NCH], [1, W]]))
                    nc.sync.dma_start(xo[:], dram_ap(x_h, base + W, [[2 * W, 128], [H * W, NCH], [1, W]]))
                ga, gb, gc, ge = (Gmaps[k][hhalf] for k in "ABCE")
                r0 = hhalf * 128
                for ch in range(NCH):
                    co = ch * W
                    ot = outp.tile([128, HH], FP, tag="o_e0lo")
                    nc.vector.scalar_tensor_tensor(ot[:], xe[:, co:co + W:2], 0.5, ga[:], MUL, ADD)
                    nc.sync.dma_start(outs["out_e0lo"].ap()[ch, r0:r0 + 128, :], ot[:])
                    ot = outp.tile([128, HH], FP, tag="o_e1lo")
                    nc.vector.scalar_tensor_tensor(ot[:], xo[:, co + 1:co + W:2], -1.0, gb[:], MUL, ADD)
                    nc.sync.dma_start(outs["out_e1lo"].ap()[ch, r0:r0 + 128, :], ot[:])
                    ot = outp.tile([128, HH], FP, tag="o_e0hi")
                    nc.vector.scalar_tensor_tensor(ot[:], xe[:, co + 1:co + W:2], -1.0, gc[:], MUL, ADD)
                    nc.sync.dma_start(outs["out_e0hi"].ap()[ch, r0:r0 + 128, :], ot[:])
                    ot = outp.tile([128, HH], FP, tag="o_e1hi")
                    nc.vector.scalar_tensor_tensor(ot[:, 0:HH - 1], xo[:, co + 2:co + W:2], 2.0, ge[:, 0:HH - 1], MUL, ADD)
                    nc.vector.scalar_tensor_tensor(ot[:, HH - 1:HH], xo[:, co:co + 1], 2.0, ge[:, HH - 1:HH], MUL, ADD)
                    nc.sync.dma_start(outs["out_e1hi"].ap()[ch, r0:r0 + 128, :], ot[:])
                xofree()
                xefree()

    nc.compile()
    return nc


def kernel(x, h, g, f):
    import numpy as np
    from concourse import bass_utils
    if "nc" not in _cache:
        _cache["nc"] = _build_nc()
        _cache["mats"] = _host_mats()
    nc = _cache["nc"]
    mats = _cache["mats"]
    x = np.ascontiguousarray(np.asarray(x, np.float32))
    in_maps = []
    for i in range(NCORES):
        m = {"x": x[i]}
        m.update(mats)
        in_maps.append(m)
    res = bass_utils.run_bass_kernel_spmd(nc, in_maps, core_ids=list(range(NCORES)))

    def stack(nm):
        return np.stack([res.results[i][nm] for i in range(NCORES)], axis=0)

    return (stack("out_c"), stack("out_e1lo"), stack("out_e0lo"),
            stack("out_e1hi"), stack("out_e0hi"))


# revision 19
# speedup vs baseline: 4.0177x; 1.0859x over previous
import numpy as np

SQ2 = 2.0 ** 0.5
H = W = 512
HH = 256
NCH = 8
NCORES = 8

_cache = {}


def _filters():
    hh = np.array([0.037828455506995, -0.02384946501938, -0.11062440441842, 0.37740285561265], np.float64)
    h = np.concatenate([hh, [0.8526986790094], hh[::-1]])
    gg = np.array([-0.064538882628938, -0.040689417609558, 0.41809227322221], np.float64)
    g = np.concatenate([gg, [0.78848561640566], gg[::-1]])
    v = np.array([0.63, -0.193, 0.0972, -0.0526, 0.0272, -0.0144], np.float64)
    f = np.concatenate([v[::-1], v])
    f[::2] = -f[::2]
    return h, g, f


def _host_mats():
    h, g, f = _filters()
    BhP = np.zeros((520, 256))
    for i in range(256):
        for p in (0, 1):
            r = 2 * i + p
            for u in range(9):
                BhP[r + u, i] += 0.5 * h[u]
    GU = np.zeros((260, 512))
    for r in range(512):
        for u in range(7):
            al = r + u - 3
            if al % 2 == 0:
                GU[al // 2 + 2, r] += g[u]
    Bf256 = np.zeros((267, 256))
    for o in range(256):
        for u in range(12):
            Bf256[o + u, o] = f[u]
    Bf512 = np.zeros((523, 512))
    for o in range(512):
        for u in range(12):
            Bf512[o + u, o] = f[u]
    PI = np.concatenate([np.arange(0, 256, 2), np.arange(256, 512, 2),
                         np.arange(1, 256, 2), np.arange(257, 512, 2)])
    Ah = np.zeros((512, 256))
    for k in range(520):
        Ah[(k - 4) % 512] += BhP[k]
    Ag = np.zeros((256, 512))
    for k in range(260):
        Ag[(k - 2) % 256] += GU[k]
    f32 = np.float32
    return {
        "Ah_r": Ah[PI].astype(f32), "Ah_c": Ah.astype(f32),
        "Ag_r": (8.0 * Ag[:, PI]).astype(f32), "Ag_c": Ag.astype(f32),
        "Bf256": Bf256.astype(f32), "Bf512": Bf512.astype(f32),
        "ident": np.eye(128, dtype=f32),
    }


def _build_nc(debug=False):
    import concourse.bass as bass
    import concourse.bacc as bacc
    import concourse.mybir as mybir
    from concourse import tile

    FP = mybir.dt.float32
    nc = bacc.Bacc("TRN2", target_bir_lowering=False, debug=False, num_devices=NCORES)
    AP = bass.AP
    MUL = mybir.AluOpType.mult
    ADD = mybir.AluOpType.add

    x_h = nc.dram_tensor("x", [NCH, H, W], FP, kind="ExternalInput")
    mat_hs = {}
    for nm, shp in [("Ah_r", (512, 256)), ("Ah_c", (512, 256)), ("Ag_r", (256, 512)),
                    ("Ag_c", (256, 512)), ("Bf256", (267, 256)), ("Bf512", (523, 512)),
                    ("ident", (128, 128))]:
        mat_hs[nm] = nc.dram_tensor(nm, list(shp), FP, kind="ExternalInput")
    outs = {}
    for nm in ["out_c", "out_e1lo", "out_e0lo", "out_e1hi", "out_e0hi"]:
        outs[nm] = nc.dram_tensor(nm, [NCH, HH, HH], FP, kind="ExternalOutput")
    dbg = {}
    if debug:
        for nm, shp in [("dMp", (512, 512)), ("dG0", (128, 1046)), ("dS1n", (256, 512)),
                        ("dT1e", (256, 256)), ("dT1o", (256, 256)), ("dX0", (256, 512)),
                        ("dS2n", (256, 512)), ("dXX", (256, 512)), ("dH0", (128, 534)),
                        ("dS3n", (256, 256)), ("dP0B", (256, 256)), ("dE0", (256, 256)),
                        ("dS4n", (256, 256)), ("dGA", (256, 256)), ("dGB", (256, 256)),
                        ("dGC", (256, 256)), ("dGE", (256, 256))]:
            dbg[nm] = nc.dram_tensor(nm, list(shp), FP, kind="ExternalOutput")

    WD2, ND2 = 790, 789
    D2 = nc.dram_tensor("D2", [ND2, WD2], FP, kind="Internal")
    X0d = nc.dram_tensor("X0d", [HH, W], FP, kind="Internal")
    E1T = nc.dram_tensor("E1T", [W, W], FP, kind="Internal")
    E2T = nc.dram_tensor("E2T", [W, W], FP, kind="Internal")
    WEX, NEX = 538, 547
    EXXT = nc.dram_tensor("EXXT", [NEX, WEX], FP, kind="Internal")
    E0d = nc.dram_tensor("E0d", [HH, HH], FP, kind="Internal")

    def dram_ap(hd, off, dims):
        return AP(hd, off, [list(d) for d in dims])

    _mats_np = _host_mats()

    with tile.TileContext(nc) as tc:
        import contextlib
        ctx = contextlib.ExitStack()
        with ctx:
            cpool = ctx.enter_context(tc.tile_pool(name="consts", bufs=1))
            keep = ctx.enter_context(tc.tile_pool(name="keep", bufs=1))
            tmp = ctx.enter_context(tc.tile_pool(name="tmp", bufs=1))
            outp = ctx.enter_context(tc.tile_pool(name="outp", bufs=2))
            psum = ctx.enter_context(tc.tile_pool(name="ps", bufs=4, space="PSUM"))

            def load_chunks(nm, rowsets, M, tag):
                tl = []
                arr = _mats_np[nm]
                for ci, rows in enumerate(rowsets):
                    kk = sum(r1 - r0 for (r0, r1) in rows)
                    t = cpool.tile([kk, M], FP, tag=f"m_{tag}_{ci}")
                    p = 0
                    for (r0, r1) in rows:
                        nc.sync.dma_start(t[p:p + (r1 - r0), :], mat_hs[nm].ap()[r0:r1, :])
                        p += r1 - r0
                    chunk_np = np.concatenate([arr[r0:r1] for (r0, r1) in rows], axis=0)
                    tl.append((kk, t, chunk_np))
                return tl

            nat4 = [[(0, 128)], [(128, 256)], [(256, 384)], [(384, 512)]]
            Ah_r_t = load_chunks("Ah_r", nat4, 256, "ahr")
            Ah_c_t = load_chunks("Ah_c", nat4, 256, "ahc")
            nat2 = [[(0, 128)], [(128, 256)]]
            Ag_r_t = load_chunks("Ag_r", nat2, 512, "agr")
            Ag_c_t = load_chunks("Ag_c", nat2, 512, "agc")
            perm267 = [[(6, 134)], [(134, 262)], [(262, 267), (0, 6)]]
            nat267 = [[(0, 128)], [(128, 256)], [(256, 267)]]
            Bf256p_t = load_chunks("Bf256", perm267, 256, "bfp")
            Bf256n_t = load_chunks("Bf256", nat267, 256, "bfn")
            nat523 = [[(0, 128)], [(128, 256)], [(256, 384)], [(384, 512)], [(512, 523)]]
            Bf512_t = load_chunks("Bf512", nat523, 512, "bf5")
            identt = cpool.tile([128, 128], FP, tag="ident")
            nc.sync.dma_start(identt[:], mat_hs["ident"].ap()[:, :])

            _cpctr = [0]

            def psum_copy(dst, src):
                _cpctr[0] += 1
                if _cpctr[0] % 2 == 0:
                    nc.scalar.activation(dst, src, mybir.ActivationFunctionType.Copy)
                else:
                    nc.vector.tensor_copy(dst, src)

            def transpose_tiles(src_tiles, R, C, pool, tag, shared=False):
                # src_tiles: list (r0, nr, tile[nr, C]) covering [R, C] -> tiles of [C, R]
                outt = []
                for c0 in range(0, C, 128):
                    cw = min(128, C - c0)
                    if shared:
                        t = pool.tile([cw, 512], FP, tag=f"{tag}_{c0}")
                    else:
                        t = pool.tile([cw, R], FP, tag=f"{tag}_{c0}")
                    for (r0, nr, st) in src_tiles:
                        ps = psum.tile([cw, nr], FP, tag="ps")
                        nc.tensor.transpose(ps[:, :], st[:, c0:c0 + cw], identt[:nr, :nr])
                        psum_copy(t[:, r0:r0 + nr], ps[:, :])
                    outt.append((c0, cw, t))
                return outt

            def band_pass(in_specs, mat_tiles, M, N, pool, tag, shared=False):
                # out[m, n] = sum_k mat[k, m] * in[k, n]
                outt = []
                for m0 in range(0, M, 128):
                    mw = min(128, M - m0)
                    if shared:
                        t = pool.tile([mw, 523], FP, tag=f"{tag}_{m0}")
                    else:
                        t = pool.tile([mw, N], FP, tag=f"{tag}_{m0}")
                    for n0 in range(0, N, 512):
                        nw = min(512, N - n0)
                        ps = psum.tile([mw, nw], FP, tag="ps")
                        active = [i for i, (_, _, cnp) in enumerate(mat_tiles)
                                  if np.any(cnp[:, m0:m0 + mw])]
                        assert active
                        for ai, ki in enumerate(active):
                            (kk, fn) = in_specs[ki]
                            (mkk, mt, _) = mat_tiles[ki]
                            assert kk == mkk
                            nc.tensor.matmul(ps[:, :], mt[:, m0:m0 + mw], fn(n0, nw),
                                             start=(ai == 0), stop=(ai == len(active) - 1))
                        psum_copy(t[:, n0:n0 + nw], ps[:, :])
                    outt.append((m0, mw, t))
                return outt

            def specs_of(tiles):
                return [(nr, (lambda t: (lambda n0, nw: t[:, n0:n0 + nw]))(t)) for (_, nr, t) in tiles]

            def conv2v(in_specs, mats_row, mats_col, Mr, Cp):
                # pass1 [Mr<=256, Cp], transpose, pass2 -> [Mc=col-mat-M, Mr] (transposed result)
                p1 = band_pass(in_specs, mats_row, Mr, Cp, tmp, "cvp1", shared=True)
                p1t = transpose_tiles(p1, Mr, Cp, tmp, "cvt1", shared=True)
                Mc = 512 if mats_col is Bf512_t else (512 if len(mats_col) == 2 else 256)
                return band_pass(specs_of(p1t), mats_col, Mc, Mr, tmp, "cvp2", shared=True)

            def pad_per_from_dram(hd, R, C, ru, cl, Rp, Cp, tag, qper=False):
                tiles = []
                for k0 in range(0, Rp, 128):
                    kk = min(128, Rp - k0)
                    t = tmp.tile([kk, 523], FP, tag=f"{tag}_{k0}")
                    k = k0
                    while k < k0 + kk:
                        a = k - ru
                        band = 0 if 0 <= a < R else (-1 if a < 0 else 1)
                        if band == -1:
                            run = min(k0 + kk - k, -a)
                        elif band == 0:
                            run = min(k0 + kk - k, R - a)
                        else:
                            run = k0 + kk - k
                        sr = a % R
                        rot = (C // 2) if (qper and band != 0) else 0
                        c = 0
                        while c < Cp:
                            sc = (c - cl + rot) % C
                            seg = min(Cp - c, C - sc)
                            nc.sync.dma_start(
                                t[k - k0:k - k0 + run, c:c + seg],
                                dram_ap(hd, sr * C + sc, [[C, run], [1, seg]]))
                            c += seg
                        k += run
                    tiles.append((k0, kk, t))
                return tiles

            # ========== stage 1: load x (parity megas) + channel sum ==========
            Xp4 = [None] * 4
            bases = [(0, 0), (0, 1), (1, 0), (1, 1)]
            for jp in (0, 2):
                megas = []
                for j in (jp, jp + 1):
                    par, hhalf = bases[j]
                    mg, mgfree = tc.tile([128, NCH * W], FP, name=f"xsum_mega_{j}")
                    base = hhalf * 2 * 128 * W + par * W
                    nc.sync.dma_start(mg[:], dram_ap(x_h, base, [[2 * W, 128], [H * W, NCH], [1, W]]))
                    megas.append((mg, mgfree))
                for idx, j in enumerate((jp, jp + 1)):
                    mg = megas[idx][0]
                    acc = keep.tile([128, W], FP, tag=f"Xp_{j}")
                    nc.vector.scalar_tensor_tensor(acc[:], mg[:, 0:W], 1.0, mg[:, W:2 * W], MUL, ADD)
                    for ch in range(2, NCH):
                        nc.vector.scalar_tensor_tensor(acc[:], mg[:, ch * W:(ch + 1) * W], 1.0, acc[:], MUL, ADD)
                    Xp4[j] = (j * 128, 128, acc)
                megas[1][1]()
                megas[0][1]()

            # ========== stage 2: cA ==========
            p2 = conv2v(specs_of(Xp4), Ah_r_t, Ah_c_t, 256, 512)
            cAt = transpose_tiles(p2, 256, 256, keep, "cA")
            for (r0, nr, t) in cAt:
                for ch in range(NCH):
                    nc.sync.dma_start(outs["out_c"].ap()[ch, r0:r0 + nr, :], t[:])

            # ========== stage 3: M (rows in PI order) ==========
            m2 = conv2v(specs_of(cAt), Ag_r_t, Ag_c_t, 512, 256)
            Mp = transpose_tiles(m2, 512, 512, keep, "Mp")

            if debug:
                for j in range(4):
                    nc.sync.dma_start(dbg["dMp"].ap()[j * 128:(j + 1) * 128, :], Mp[j][2][:])
            # ========== stage 4: Dsum; write D2 ==========
            colsegs = [(0, 500, 12), (12, 0, 512), (524, 0, 266)]
            for j, off in enumerate([0, 256, 1, 257]):
                d = tmp.tile([128, W], FP, tag=f"Ds_{j % 2}")
                nc.vector.scalar_tensor_tensor(d[:], Mp[j][2][:], -8.0, Xp4[j][2][:], MUL, ADD)
                for (dc, sc, seg) in colsegs:
                    nc.sync.dma_start(
                        dram_ap(D2, (10 + off) * WD2 + dc, [[2 * WD2, 128], [1, seg]]),
                        d[:, sc:sc + seg])
                nb = 128 if off in (0, 1) else (6 if off == 256 else 5)
                for (dc, sc, seg) in colsegs:
                    nc.sync.dma_start(
                        dram_ap(D2, (522 + off) * WD2 + dc, [[2 * WD2, nb], [1, seg]]),
                        d[0:nb, sc:sc + seg])
                # band C: D2 rows 0..9 = Dsum rows 502..511 (u = r - 1024)
                if off in (256, 257):
                    # tile rows p in [123,128) -> r = off + 2p in [502, 511]; D2 row r - 502
                    for (dc, sc, seg) in colsegs:
                        nc.sync.dma_start(
                            dram_ap(D2, (off - 256) * WD2 + dc, [[2 * WD2, 5], [1, seg]]),
                            d[123:128, sc:sc + seg])

            # early stage-10 x loads for half 0 (fills DMA idle during conv phases)
            xe0, xe0free = tc.tile([128, NCH * W], FP, name="xout_e_0")
            xo0, xo0free = tc.tile([128, NCH * W], FP, name="xout_o_0")
            nc.sync.dma_start(xe0[:], dram_ap(x_h, 0, [[2 * W, 128], [H * W, NCH], [1, W]]))
            nc.sync.dma_start(xo0[:], dram_ap(x_h, W, [[2 * W, 128], [H * W, NCH], [1, W]]))

            # ========== stage 5: P-pair gathers; S1; E1T; X0; Tp1 ==========
            def ppair_tile(a0, npart, dst):
                nc.sync.dma_start(dst, dram_ap(
                    D2, (528 + a0) * WD2 + 6 + a0, [[WD2 + 1, npart], [-(WD2 - 1), 523], [1, 2]]))

            G0 = keep.tile([128, 1046], FP, tag="G0")
            G1 = keep.tile([128, 1046], FP, tag="G1")
            G2 = keep.tile([11, 1046], FP, tag="G2")
            ppair_tile(0, 128, G0[:])
            ppair_tile(128, 128, G1[:])
            ppair_tile(256, 5, G2[0:5, :])
            ppair_tile(-6, 6, G2[5:11, :])

            if debug:
                nc.sync.dma_start(dbg["dG0"].ap()[:, :], G0[:])

            def pair_specs(gtiles):
                return [(nk, (lambda t: (lambda n0, nw: t[:, 2 * n0 + 1:2 * (n0 + nw):2]))(t))
                        for (nk, t) in gtiles]

            S1T = conv2v(pair_specs([(128, G0), (128, G1), (11, G2)]), Bf256p_t, Bf512_t, 256, 523)
            for (c0, nr, t) in S1T:
                nc.sync.dma_start(dram_ap(E1T, ((c0 + 256) % 512) * W, [[W, 128], [1, 256]]), t[:, 0:256])
                nc.sync.dma_start(dram_ap(E1T, c0 * W + 256, [[W, 128], [1, 256]]), t[:, 0:256])
            S1n = transpose_tiles(S1T, 512, 256, keep, "S1n")
            if debug:
                for (r0, nr, t) in S1n:
                    nc.sync.dma_start(dbg["dS1n"].ap()[r0:r0 + nr, :], t[:, 0:W])
            X0t = []
            for (r0, nr, s1) in S1n:
                g = (G0 if r0 == 0 else G1)
                x0 = keep.tile([128, W], FP, tag=f"X0_{r0}")
                nc.vector.scalar_tensor_tensor(x0[:], s1[:, 0:W], -8.0, g[:, 12:12 + 2 * W:2], MUL, ADD)
                nc.vector.tensor_scalar_mul(x0[:], x0[:], 1.0 / SQ2)
                nc.sync.dma_start(X0d.ap()[r0:r0 + nr, :], x0[:])
                X0t.append((r0, nr, x0))

            # T-pair gather partitioned by b (positive partition stride):
            # TpT[b, 2a+e] = E1T[(b-a+256)*512 + (a+b+e)]
            def tpair(hd, b0, dst):
                nc.sync.dma_start(dst, dram_ap(
                    hd, (b0 + 256) * W + b0, [[W + 1, 128], [-(W - 1), 256], [1, 2]]))

            def tmaps(hd, tagpfx):
                # returns {0: [2 tiles [128a, 256b]], 1: [...]} for e=0 (Te), e=1 (To)
                tpT = []
                for b0 in (0, 128):
                    t = tmp.tile([128, 512], FP, tag=f"tpT_{b0}")
                    tpair(hd, b0, t[:])
                    tpT.append(t)
                res = {}
                for e in (0, 1):
                    cps = []
                    for bi, t in enumerate(tpT):
                        cp = tmp.tile([128, 256], FP, tag=f"tpc_{bi}")
                        nc.vector.tensor_copy(cp[:], t[:, e:512:2])
                        cps.append((bi * 128, 128, cp))
                    res[e] = transpose_tiles(cps, 256, 256, keep, f"{tagpfx}{e}")
                return res

            T1 = tmaps(E1T, "T1_")
            if debug:
                for e, nm in ((0, "dT1e"), (1, "dT1o")):
                    for (c0, cw, t) in T1[e]:
                        nc.sync.dma_start(dbg[nm].ap()[c0:c0 + cw, :], t[:])
                for (r0, nr, x0) in X0t:
                    nc.sync.dma_start(dbg["dX0"].ap()[r0:r0 + nr, :], x0[:])

            # ========== stage 6: S2; E2T; Tp2; XX; EXXT ==========
            X0p = pad_per_from_dram(X0d, HH, W, 5, 5, 267, 523, "padp", qper=True)
            S2T = conv2v(specs_of(X0p), Bf256n_t, Bf512_t, 256, 523)
            for (c0, nr, t) in S2T:
                nc.sync.dma_start(dram_ap(E2T, ((c0 + 256) % 512) * W, [[W, 128], [1, 256]]), t[:, 0:256])
                nc.sync.dma_start(dram_ap(E2T, c0 * W + 256, [[W, 128], [1, 256]]), t[:, 0:256])
            S2n = transpose_tiles(S2T, 512, 256, tmp, "S2n")
            T2 = tmaps(E2T, "T2_")

            XXt = []
            for (r0, nr, s2) in S2n:
                g = (G0 if r0 == 0 else G1)
                xx = tmp.tile([128, W], FP, tag=f"XX_{r0}")
                x0 = X0t[r0 // 128][2]
                nc.vector.scalar_tensor_tensor(xx[:], s2[:, 0:W], -8.0, x0[:], MUL, ADD)
                nc.vector.scalar_tensor_tensor(xx[:], g[:, 13:13 + 2 * W:2], -SQ2, xx[:], MUL, ADD)
                XXt.append((r0, nr, xx))
            XXT = transpose_tiles(XXt, 256, 512, tmp, "XXT")

            if debug:
                for (r0, nr, t) in S2n:
                    nc.sync.dma_start(dbg["dS2n"].ap()[r0:r0 + nr, :], t[:, 0:W])
                for (r0, nr, t) in XXt:
                    nc.sync.dma_start(dbg["dXX"].ap()[r0:r0 + nr, :], t[:])
            for (s0, ns, t) in XXT:
                for (k, pcol0, a0, cnt) in [(0, 16, 0, 256), (1, 272, 0, 256), (-1, 4, 244, 12), (2, 528, 0, 10)]:
                    qb0 = (280 - 256 * k + s0) % 512
                    runs = [(0, qb0, min(ns, 512 - qb0))]
                    if runs[0][2] < ns:
                        runs.append((runs[0][2], 0, ns - runs[0][2]))
                    for (srow, qs, rl) in runs:
                        for qc in (qs, qs + 512):
                            tlo = max(0, 14 - qc)
                            thi = min(rl, NEX - qc)
                            if tlo < thi:
                                nc.sync.dma_start(
                                    dram_ap(EXXT, (qc + tlo) * WEX + pcol0, [[WEX, thi - tlo], [1, cnt]]),
                                    t[srow + tlo:srow + thi, a0:a0 + cnt])

            # ========== stage 7: C-pair gathers (partitioned by j); S3; E0 ==========
            # H[jj, 2*ii+e] = EXXT[(j-i+280)*WEX + (i+j+e+16)], j = j0+p, i = ii-6
            def cpair_tile(j0, npart, dst):
                nc.sync.dma_start(dst, dram_ap(
                    EXXT, (j0 + 286) * WEX + j0 + 10, [[WEX + 1, npart], [-(WEX - 1), 267], [1, 2]]))

            H0 = keep.tile([128, 534], FP, tag="H0")
            H1 = keep.tile([128, 534], FP, tag="H1")
            H2 = keep.tile([11, 534], FP, tag="H2")
            cpair_tile(0, 128, H0[:])
            cpair_tile(128, 128, H1[:])
            cpair_tile(256, 5, H2[0:5, :])
            cpair_tile(-6, 6, H2[5:11, :])

            if debug:
                nc.sync.dma_start(dbg["dH0"].ap()[:, :], H0[:])
            # pass1 contracts j (partition dim of H), pass2 contracts i -> S3 natural
            S3n = conv2v(pair_specs([(128, H0), (128, H1), (11, H2)]), Bf256p_t, Bf256n_t, 256, 267)
            # P0B via strided copy + PE transpose: P0BT[b, a] = H_b[:, 12+2a]
            p0bt = []
            for bi, hgt in enumerate((H0, H1)):
                cp = tmp.tile([128, 256], FP, tag=f"tpc_{bi}")
                nc.vector.tensor_copy(cp[:], hgt[:, 12:12 + 2 * HH:2])
                p0bt.append((bi * 128, 128, cp))
            P0Bn = transpose_tiles(p0bt, 256, 256, tmp, "p0bn")
            if debug:
                for (r0, nr, t) in S3n:
                    nc.sync.dma_start(dbg["dS3n"].ap()[r0:r0 + nr, :], t[:, 0:HH])
                for (c0, cw, t) in P0Bn:
                    nc.sync.dma_start(dbg["dP0B"].ap()[c0:c0 + cw, :], t[:])
            for ((r0, nr, s3), (_, _, p0b)) in zip(S3n, P0Bn):
                e0 = outp.tile([128, HH], FP, tag="E0w")
                nc.vector.scalar_tensor_tensor(e0[:], s3[:, 0:HH], -16.0, p0b[:, 0:HH], MUL, ADD)
                nc.vector.tensor_scalar_mul(e0[:], e0[:], 1.0 / SQ2)
                nc.sync.dma_start(E0d.ap()[r0:r0 + nr, :], e0[:])
                if debug:
                    nc.sync.dma_start(dbg["dE0"].ap()[r0:r0 + nr, :], e0[:])

            # ========== stage 8: S4 ==========
            E0p = pad_per_from_dram(E0d, HH, HH, 5, 5, 267, 267, "padp")
            S4T = conv2v(specs_of(E0p), Bf256n_t, Bf256n_t, 256, 267)
            S4n = transpose_tiles(S4T, 256, 256, keep, "S4n")

            if debug:
                dd = np_none = None
                for (r0, nr, t) in S4n:
                    nc.sync.dma_start(dbg["dS4n"].ap()[r0:r0 + nr, :], t[:, 0:HH])
            # ========== stage 9: broadcast maps ==========
            Gmaps = {k: [] for k in "ABCE"}
            for ti in range(2):
                r0 = ti * 128
                s3 = S3n[ti][2]
                s4 = S4n[ti][2]
                t1e = T1[0][ti][2]
                t1o = T1[1][ti][2]
                t2e = T2[0][ti][2]
                t2o = T2[1][ti][2]
                me = Mp[ti][2]
                mo = Mp[2 + ti][2]
                ga = keep.tile([128, HH], FP, tag=f"GA_{r0}")
                nc.vector.tensor_scalar_mul(ga[:], s3[:, 0:HH], -1.0 / SQ2)
                nc.vector.scalar_tensor_tensor(ga[:], t1e[:, 0:HH], -0.5, ga[:], MUL, ADD)
                nc.vector.scalar_tensor_tensor(ga[:], me[:, 0:W:2], -0.5, ga[:], MUL, ADD)
                gb = keep.tile([128, HH], FP, tag=f"GB_{r0}")
                nc.vector.scalar_tensor_tensor(gb[:], s4[:, 0:HH], -1.0, t1o[:, 0:HH], MUL, ADD)
                nc.vector.scalar_tensor_tensor(gb[:], mo[:, 1:W:2], 1.0, gb[:], MUL, ADD)
                gc = keep.tile([128, HH], FP, tag=f"GC_{r0}")
                nc.vector.tensor_scalar_mul(gc[:], s3[:, 0:HH], -1.0 / SQ2)
                nc.vector.scalar_tensor_tensor(gc[:], t2e[:, 0:HH], -1.0 / SQ2, gc[:], MUL, ADD)
                nc.vector.scalar_tensor_tensor(gc[:], me[:, 1:W:2], 1.0, gc[:], MUL, ADD)
                ge = keep.tile([128, HH], FP, tag=f"GE_{r0}")
                nc.vector.tensor_scalar_mul(ge[:], s4[:, 0:HH], -1.0)
                nc.vector.scalar_tensor_tensor(ge[:], t2o[:, 0:HH], SQ2, ge[:], MUL, ADD)
                nc.vector.scalar_tensor_tensor(ge[:, 0:HH - 1], mo[:, 2:W:2], -2.0, ge[:, 0:HH - 1], MUL, ADD)
                nc.vector.scalar_tensor_tensor(ge[:, HH - 1:HH], mo[:, 0:1], -2.0, ge[:, HH - 1:HH], MUL, ADD)
                Gmaps["A"].append(ga)
                Gmaps["B"].append(gb)
                Gmaps["C"].append(gc)
                Gmaps["E"].append(ge)

            if debug:
                for ti, r0 in ((0, 0), (1, 128)):
                    for gk, nm in (("A", "dGA"), ("B", "dGB"), ("C", "dGC"), ("E", "dGE")):
                        nc.sync.dma_start(dbg[nm].ap()[r0:r0 + 128, :], Gmaps[gk][ti][:])
            # ========== stage 10: per-channel outputs ==========
            for hhalf in range(2):
                if hhalf == 0:
                    xe, xo = xe0, xo0
                    xefree, xofree = xe0free, xo0free
                else:
                    xe, xefree = tc.tile([128, NCH * W], FP, name="xout_e_1")
                    xo, xofree = tc.tile([128, NCH * W], FP, name="xout_o_1")
                    base = hhalf * 2 * 128 * W
                    nc.sync.dma_start(xe[:], dram_ap(x_h, base, [[2 * W, 128], [H * W, NCH], [1, W]]))
                    nc.sync.dma_start(xo[:], dram_ap(x_h, base + W, [[2 * W, 128], [H * W, NCH], [1, W]]))
                ga, gb, gc, ge = (Gmaps[k][hhalf] for k in "ABCE")
                r0 = hhalf * 128
                for ch in range(NCH):
                    co = ch * W
                    ot = outp.tile([128, HH], FP, tag="o_e0lo")
                    nc.vector.scalar_tensor_tensor(ot[:], xe[:, co:co + W:2], 0.5, ga[:], MUL, ADD)
                    nc.sync.dma_start(outs["out_e0lo"].ap()[ch, r0:r0 + 128, :], ot[:])
                    ot = outp.tile([128, HH], FP, tag="o_e1lo")
                    nc.vector.scalar_tensor_tensor(ot[:], xo[:, co + 1:co + W:2], -1.0, gb[:], MUL, ADD)
                    nc.sync.dma_start(outs["out_e1lo"].ap()[ch, r0:r0 + 128, :], ot[:])
                    ot = outp.tile([128, HH], FP, tag="o_e0hi")
                    nc.vector.scalar_tensor_tensor(ot[:], xe[:, co + 1:co + W:2], -1.0, gc[:], MUL, ADD)
                    nc.sync.dma_start(outs["out_e0hi"].ap()[ch, r0:r0 + 128, :], ot[:])
                    ot = outp.tile([128, HH], FP, tag="o_e1hi")
                    nc.vector.scalar_tensor_tensor(ot[:, 0:HH - 1], xo[:, co + 2:co + W:2], 2.0, ge[:, 0:HH - 1], MUL, ADD)
                    nc.vector.scalar_tensor_tensor(ot[:, HH - 1:HH], xo[:, co:co + 1], 2.0, ge[:, HH - 1:HH], MUL, ADD)
                    nc.sync.dma_start(outs["out_e1hi"].ap()[ch, r0:r0 + 128, :], ot[:])
                xofree()
                xefree()

    nc.compile()
    return nc


def kernel(x, h, g, f):
    import numpy as np
    from concourse import bass_utils
    if "nc" not in _cache:
        _cache["nc"] = _build_nc()
        _cache["mats"] = _host_mats()
    nc = _cache["nc"]
    mats = _cache["mats"]
    x = np.ascontiguousarray(np.asarray(x, np.float32))
    in_maps = []
    for i in range(NCORES):
        m = {"x": x[i]}
        m.update(mats)
        in_maps.append(m)
    res = bass_utils.run_bass_kernel_spmd(nc, in_maps, core_ids=list(range(NCORES)))

    def stack(nm):
        return np.stack([res.results[i][nm] for i in range(NCORES)], axis=0)

    return (stack("out_c"), stack("out_e1lo"), stack("out_e0lo"),
            stack("out_e1hi"), stack("out_e0hi"))


# revision 20
# speedup vs baseline: 4.1075x; 1.0224x over previous
import numpy as np

SQ2 = 2.0 ** 0.5
H = W = 512
HH = 256
NCH = 8
NCORES = 8

_cache = {}


def _filters():
    hh = np.array([0.037828455506995, -0.02384946501938, -0.11062440441842, 0.37740285561265], np.float64)
    h = np.concatenate([hh, [0.8526986790094], hh[::-1]])
    gg = np.array([-0.064538882628938, -0.040689417609558, 0.41809227322221], np.float64)
    g = np.concatenate([gg, [0.78848561640566], gg[::-1]])
    v = np.array([0.63, -0.193, 0.0972, -0.0526, 0.0272, -0.0144], np.float64)
    f = np.concatenate([v[::-1], v])
    f[::2] = -f[::2]
    return h, g, f


def _host_mats():
    h, g, f = _filters()
    BhP = np.zeros((520, 256))
    for i in range(256):
        for p in (0, 1):
            r = 2 * i + p
            for u in range(9):
                BhP[r + u, i] += 0.5 * h[u]
    GU = np.zeros((260, 512))
    for r in range(512):
        for u in range(7):
            al = r + u - 3
            if al % 2 == 0:
                GU[al // 2 + 2, r] += g[u]
    Bf256 = np.zeros((267, 256))
    for o in range(256):
        for u in range(12):
            Bf256[o + u, o] = f[u]
    Bf512 = np.zeros((523, 512))
    for o in range(512):
        for u in range(12):
            Bf512[o + u, o] = f[u]
    PI = np.concatenate([np.arange(0, 256, 2), np.arange(256, 512, 2),
                         np.arange(1, 256, 2), np.arange(257, 512, 2)])
    Ah = np.zeros((512, 256))
    for k in range(520):
        Ah[(k - 4) % 512] += BhP[k]
    Ag = np.zeros((256, 512))
    for k in range(260):
        Ag[(k - 2) % 256] += GU[k]
    f32 = np.float32
    return {
        "Ah_r": Ah[PI].astype(f32), "Ah_c": Ah.astype(f32),
        "Ag_r": (8.0 * Ag[:, PI]).astype(f32), "Ag_c": Ag.astype(f32),
        "Bf256": Bf256.astype(f32), "Bf512": Bf512.astype(f32),
        "ident": np.eye(128, dtype=f32),
    }


def _build_nc(debug=False):
    import concourse.bass as bass
    import concourse.bacc as bacc
    import concourse.mybir as mybir
    from concourse import tile

    FP = mybir.dt.float32
    nc = bacc.Bacc("TRN2", target_bir_lowering=False, debug=False, num_devices=NCORES)
    AP = bass.AP
    MUL = mybir.AluOpType.mult
    ADD = mybir.AluOpType.add

    x_h = nc.dram_tensor("x", [NCH, H, W], FP, kind="ExternalInput")
    mat_hs = {}
    for nm, shp in [("Ah_r", (512, 256)), ("Ah_c", (512, 256)), ("Ag_r", (256, 512)),
                    ("Ag_c", (256, 512)), ("Bf256", (267, 256)), ("Bf512", (523, 512)),
                    ("ident", (128, 128))]:
        mat_hs[nm] = nc.dram_tensor(nm, list(shp), FP, kind="ExternalInput")
    outs = {}
    for nm in ["out_c", "out_e1lo", "out_e0lo", "out_e1hi", "out_e0hi"]:
        outs[nm] = nc.dram_tensor(nm, [NCH, HH, HH], FP, kind="ExternalOutput")
    dbg = {}
    if debug:
        for nm, shp in [("dMp", (512, 512)), ("dG0", (128, 1046)), ("dS1n", (256, 512)),
                        ("dT1e", (256, 256)), ("dT1o", (256, 256)), ("dX0", (256, 512)),
                        ("dS2n", (256, 512)), ("dXX", (256, 512)), ("dH0", (128, 534)),
                        ("dS3n", (256, 256)), ("dP0B", (256, 256)), ("dE0", (256, 256)),
                        ("dS4n", (256, 256)), ("dGA", (256, 256)), ("dGB", (256, 256)),
                        ("dGC", (256, 256)), ("dGE", (256, 256))]:
            dbg[nm] = nc.dram_tensor(nm, list(shp), FP, kind="ExternalOutput")

    WD2, ND2 = 790, 789
    D2 = nc.dram_tensor("D2", [ND2, WD2], FP, kind="Internal")
    X0d = nc.dram_tensor("X0d", [HH, W], FP, kind="Internal")
    E1T = nc.dram_tensor("E1T", [W, W], FP, kind="Internal")
    E2T = nc.dram_tensor("E2T", [W, W], FP, kind="Internal")
    WEX, NEX = 538, 547
    EXXT = nc.dram_tensor("EXXT", [NEX, WEX], FP, kind="Internal")
    E0d = nc.dram_tensor("E0d", [HH, HH], FP, kind="Internal")

    def dram_ap(hd, off, dims):
        return AP(hd, off, [list(d) for d in dims])

    _mats_np = _host_mats()

    with tile.TileContext(nc) as tc:
        import contextlib
        ctx = contextlib.ExitStack()
        with ctx:
            cpool = ctx.enter_context(tc.tile_pool(name="consts", bufs=1))
            keep = ctx.enter_context(tc.tile_pool(name="keep", bufs=1))
            tmp = ctx.enter_context(tc.tile_pool(name="tmp", bufs=1))
            outp = ctx.enter_context(tc.tile_pool(name="outp", bufs=2))
            psum = ctx.enter_context(tc.tile_pool(name="ps", bufs=4, space="PSUM"))

            def load_chunks(nm, rowsets, M, tag):
                tl = []
                arr = _mats_np[nm]
                for ci, rows in enumerate(rowsets):
                    kk = sum(r1 - r0 for (r0, r1) in rows)
                    t = cpool.tile([kk, M], FP, tag=f"m_{tag}_{ci}")
                    p = 0
                    for (r0, r1) in rows:
                        nc.sync.dma_start(t[p:p + (r1 - r0), :], mat_hs[nm].ap()[r0:r1, :])
                        p += r1 - r0
                    chunk_np = np.concatenate([arr[r0:r1] for (r0, r1) in rows], axis=0)
                    tl.append((kk, t, chunk_np))
                return tl

            nat4 = [[(0, 128)], [(128, 256)], [(256, 384)], [(384, 512)]]
            Ah_r_t = load_chunks("Ah_r", nat4, 256, "ahr")
            Ah_c_t = load_chunks("Ah_c", nat4, 256, "ahc")
            nat2 = [[(0, 128)], [(128, 256)]]
            Ag_r_t = load_chunks("Ag_r", nat2, 512, "agr")
            Ag_c_t = load_chunks("Ag_c", nat2, 512, "agc")
            perm267 = [[(6, 134)], [(134, 262)], [(262, 267), (0, 6)]]
            nat267 = [[(0, 128)], [(128, 256)], [(256, 267)]]
            Bf256p_t = load_chunks("Bf256", perm267, 256, "bfp")
            Bf256n_t = load_chunks("Bf256", nat267, 256, "bfn")
            nat523 = [[(0, 128)], [(128, 256)], [(256, 384)], [(384, 512)], [(512, 523)]]
            Bf512_t = load_chunks("Bf512", nat523, 512, "bf5")
            identt = cpool.tile([128, 128], FP, tag="ident")
            nc.sync.dma_start(identt[:], mat_hs["ident"].ap()[:, :])

            _cpctr = [0]

            def psum_copy(dst, src):
                _cpctr[0] += 1
                if _cpctr[0] % 2 == 0:
                    nc.scalar.activation(dst, src, mybir.ActivationFunctionType.Copy)
                else:
                    nc.vector.tensor_copy(dst, src)

            def transpose_tiles(src_tiles, R, C, pool, tag, shared=False):
                # src_tiles: list (r0, nr, tile[nr, C]) covering [R, C] -> tiles of [C, R]
                outt = []
                for c0 in range(0, C, 128):
                    cw = min(128, C - c0)
                    if shared:
                        t = pool.tile([cw, 512], FP, tag=f"{tag}_{c0}")
                    else:
                        t = pool.tile([cw, R], FP, tag=f"{tag}_{c0}")
                    for (r0, nr, st) in src_tiles:
                        ps = psum.tile([cw, nr], FP, tag="ps")
                        nc.tensor.transpose(ps[:, :], st[:, c0:c0 + cw], identt[:nr, :nr])
                        psum_copy(t[:, r0:r0 + nr], ps[:, :])
                    outt.append((c0, cw, t))
                return outt

            def band_pass(in_specs, mat_tiles, M, N, pool, tag, shared=False):
                # out[m, n] = sum_k mat[k, m] * in[k, n]
                outt = []
                for m0 in range(0, M, 128):
                    mw = min(128, M - m0)
                    if shared:
                        t = pool.tile([mw, 523], FP, tag=f"{tag}_{m0}")
                    else:
                        t = pool.tile([mw, N], FP, tag=f"{tag}_{m0}")
                    for n0 in range(0, N, 512):
                        nw = min(512, N - n0)
                        ps = psum.tile([mw, nw], FP, tag="ps")
                        active = [i for i, (_, _, cnp) in enumerate(mat_tiles)
                                  if np.any(cnp[:, m0:m0 + mw])]
                        assert active
                        for ai, ki in enumerate(active):
                            (kk, fn) = in_specs[ki]
                            (mkk, mt, _) = mat_tiles[ki]
                            assert kk == mkk
                            nc.tensor.matmul(ps[:, :], mt[:, m0:m0 + mw], fn(n0, nw),
                                             start=(ai == 0), stop=(ai == len(active) - 1))
                        psum_copy(t[:, n0:n0 + nw], ps[:, :])
                    outt.append((m0, mw, t))
                return outt

            def specs_of(tiles):
                return [(nr, (lambda t: (lambda n0, nw: t[:, n0:n0 + nw]))(t)) for (_, nr, t) in tiles]

            def conv2v(in_specs, mats_row, mats_col, Mr, Cp):
                # pass1 [Mr<=256, Cp], transpose, pass2 -> [Mc=col-mat-M, Mr] (transposed result)
                p1 = band_pass(in_specs, mats_row, Mr, Cp, tmp, "cvp1", shared=True)
                p1t = transpose_tiles(p1, Mr, Cp, tmp, "cvt1", shared=True)
                Mc = 512 if mats_col is Bf512_t else (512 if len(mats_col) == 2 else 256)
                return band_pass(specs_of(p1t), mats_col, Mc, Mr, tmp, "cvp2", shared=True)

            def pad_per_from_dram(hd, R, C, ru, cl, Rp, Cp, tag, qper=False):
                tiles = []
                for k0 in range(0, Rp, 128):
                    kk = min(128, Rp - k0)
                    t = tmp.tile([kk, 523], FP, tag=f"{tag}_{k0}")
                    k = k0
                    while k < k0 + kk:
                        a = k - ru
                        band = 0 if 0 <= a < R else (-1 if a < 0 else 1)
                        if band == -1:
                            run = min(k0 + kk - k, -a)
                        elif band == 0:
                            run = min(k0 + kk - k, R - a)
                        else:
                            run = k0 + kk - k
                        sr = a % R
                        rot = (C // 2) if (qper and band != 0) else 0
                        c = 0
                        while c < Cp:
                            sc = (c - cl + rot) % C
                            seg = min(Cp - c, C - sc)
                            nc.sync.dma_start(
                                t[k - k0:k - k0 + run, c:c + seg],
                                dram_ap(hd, sr * C + sc, [[C, run], [1, seg]]))
                            c += seg
                        k += run
                    tiles.append((k0, kk, t))
                return tiles

            # ========== stage 1: load x (parity megas) + channel sum ==========
            Xp4 = [None] * 4
            bases = [(0, 0), (0, 1), (1, 0), (1, 1)]
            for jp in (0, 2):
                megas = []
                for j in (jp, jp + 1):
                    par, hhalf = bases[j]
                    mg, mgfree = tc.tile([128, NCH * W], FP, name=f"xsum_mega_{j}")
                    base = hhalf * 2 * 128 * W + par * W
                    nc.sync.dma_start(mg[:], dram_ap(x_h, base, [[2 * W, 128], [H * W, NCH], [1, W]]))
                    megas.append((mg, mgfree))
                for idx, j in enumerate((jp, jp + 1)):
                    mg = megas[idx][0]
                    acc = keep.tile([128, W], FP, tag=f"Xp_{j}")
                    nc.vector.scalar_tensor_tensor(acc[:], mg[:, 0:W], 1.0, mg[:, W:2 * W], MUL, ADD)
                    for ch in range(2, NCH):
                        nc.vector.scalar_tensor_tensor(acc[:], mg[:, ch * W:(ch + 1) * W], 1.0, acc[:], MUL, ADD)
                    Xp4[j] = (j * 128, 128, acc)
                megas[1][1]()
                megas[0][1]()

            # ========== stage 2: cA ==========
            p2 = conv2v(specs_of(Xp4), Ah_r_t, Ah_c_t, 256, 512)
            cAt = transpose_tiles(p2, 256, 256, keep, "cA")
            for (r0, nr, t) in cAt:
                for ch in range(NCH):
                    nc.sync.dma_start(outs["out_c"].ap()[ch, r0:r0 + nr, :], t[:])

            # ========== stage 3: M (rows in PI order) ==========
            m2 = conv2v(specs_of(cAt), Ag_r_t, Ag_c_t, 512, 256)
            Mp = transpose_tiles(m2, 512, 512, keep, "Mp")

            if debug:
                for j in range(4):
                    nc.sync.dma_start(dbg["dMp"].ap()[j * 128:(j + 1) * 128, :], Mp[j][2][:])
            # ========== stage 4: Dsum; write D2 ==========
            colsegs = [(0, 500, 12), (12, 0, 512), (524, 0, 266)]
            for j, off in enumerate([0, 256, 1, 257]):
                d = tmp.tile([128, W], FP, tag=f"Ds_{j % 2}")
                nc.vector.scalar_tensor_tensor(d[:], Mp[j][2][:], -8.0, Xp4[j][2][:], MUL, ADD)
                for (dc, sc, seg) in colsegs:
                    nc.sync.dma_start(
                        dram_ap(D2, (10 + off) * WD2 + dc, [[2 * WD2, 128], [1, seg]]),
                        d[:, sc:sc + seg])
                nb = 128 if off in (0, 1) else (6 if off == 256 else 5)
                for (dc, sc, seg) in colsegs:
                    nc.sync.dma_start(
                        dram_ap(D2, (522 + off) * WD2 + dc, [[2 * WD2, nb], [1, seg]]),
                        d[0:nb, sc:sc + seg])
                # band C: D2 rows 0..9 = Dsum rows 502..511 (u = r - 1024)
                if off in (256, 257):
                    # tile rows p in [123,128) -> r = off + 2p in [502, 511]; D2 row r - 502
                    for (dc, sc, seg) in colsegs:
                        nc.sync.dma_start(
                            dram_ap(D2, (off - 256) * WD2 + dc, [[2 * WD2, 5], [1, seg]]),
                            d[123:128, sc:sc + seg])

            # early stage-10 x loads for half 0 (fills DMA idle during conv phases)
            xe0, xe0free = tc.tile([128, NCH * W], FP, name="xout_e_0")
            xo0, xo0free = tc.tile([128, NCH * W], FP, name="xout_o_0")
            nc.sync.dma_start(xe0[:], dram_ap(x_h, 0, [[2 * W, 128], [H * W, NCH], [1, W]]))
            nc.sync.dma_start(xo0[:], dram_ap(x_h, W, [[2 * W, 128], [H * W, NCH], [1, W]]))

            # ========== stage 5: P-pair gathers; S1; E1T; X0; Tp1 ==========
            def ppair_tile(a0, npart, dst):
                nc.sync.dma_start(dst, dram_ap(
                    D2, (528 + a0) * WD2 + 6 + a0, [[WD2 + 1, npart], [-(WD2 - 1), 523], [1, 2]]))

            G0 = keep.tile([128, 1046], FP, tag="G0")
            G1 = keep.tile([128, 1046], FP, tag="G1")
            G2 = keep.tile([11, 1046], FP, tag="G2")
            ppair_tile(0, 128, G0[:])
            ppair_tile(128, 128, G1[:])
            ppair_tile(256, 5, G2[0:5, :])
            ppair_tile(-6, 6, G2[5:11, :])

            if debug:
                nc.sync.dma_start(dbg["dG0"].ap()[:, :], G0[:])

            def pair_specs(gtiles):
                return [(nk, (lambda t: (lambda n0, nw: t[:, 2 * n0 + 1:2 * (n0 + nw):2]))(t))
                        for (nk, t) in gtiles]

            S1T = conv2v(pair_specs([(128, G0), (128, G1), (11, G2)]), Bf256p_t, Bf512_t, 256, 523)
            for (c0, nr, t) in S1T:
                nc.sync.dma_start(dram_ap(E1T, ((c0 + 256) % 512) * W, [[W, 128], [1, 256]]), t[:, 0:256])
                nc.sync.dma_start(dram_ap(E1T, c0 * W + 256, [[W, 128], [1, 256]]), t[:, 0:256])
            S1n = transpose_tiles(S1T, 512, 256, keep, "S1n")
            if debug:
                for (r0, nr, t) in S1n:
                    nc.sync.dma_start(dbg["dS1n"].ap()[r0:r0 + nr, :], t[:, 0:W])
            X0t = []
            for (r0, nr, s1) in S1n:
                g = (G0 if r0 == 0 else G1)
                x0 = keep.tile([128, W], FP, tag=f"X0_{r0}")
                nc.vector.scalar_tensor_tensor(x0[:], s1[:, 0:W], -8.0, g[:, 12:12 + 2 * W:2], MUL, ADD)
                nc.vector.tensor_scalar_mul(x0[:], x0[:], 1.0 / SQ2)
                nc.sync.dma_start(X0d.ap()[r0:r0 + nr, :], x0[:])
                X0t.append((r0, nr, x0))

            # T-pair gather partitioned by b (positive partition stride):
            # TpT[b, 2a+e] = E1T[(b-a+256)*512 + (a+b+e)]
            def tpair(hd, b0, dst):
                nc.sync.dma_start(dst, dram_ap(
                    hd, (b0 + 256) * W + b0, [[W + 1, 128], [-(W - 1), 256], [1, 2]]))

            def tmaps(hd, tagpfx):
                # returns {0: [2 tiles [128a, 256b]], 1: [...]} for e=0 (Te), e=1 (To)
                tpT = []
                for b0 in (0, 128):
                    t = tmp.tile([128, 512], FP, tag=f"tpT_{b0}")
                    tpair(hd, b0, t[:])
                    tpT.append(t)
                res = {}
                for e in (0, 1):
                    cps = []
                    for bi, t in enumerate(tpT):
                        cp = tmp.tile([128, 256], FP, tag=f"tpc_{bi}")
                        nc.vector.tensor_copy(cp[:], t[:, e:512:2])
                        cps.append((bi * 128, 128, cp))
                    res[e] = transpose_tiles(cps, 256, 256, keep, f"{tagpfx}{e}")
                return res

            T1 = tmaps(E1T, "T1_")
            if debug:
                for e, nm in ((0, "dT1e"), (1, "dT1o")):
                    for (c0, cw, t) in T1[e]:
                        nc.sync.dma_start(dbg[nm].ap()[c0:c0 + cw, :], t[:])
                for (r0, nr, x0) in X0t:
                    nc.sync.dma_start(dbg["dX0"].ap()[r0:r0 + nr, :], x0[:])

            # ========== stage 6: S2; E2T; Tp2; XX; EXXT ==========
            X0p = pad_per_from_dram(X0d, HH, W, 5, 5, 267, 523, "padp", qper=True)
            S2T = conv2v(specs_of(X0p), Bf256n_t, Bf512_t, 256, 523)
            for (c0, nr, t) in S2T:
                nc.sync.dma_start(dram_ap(E2T, ((c0 + 256) % 512) * W, [[W, 128], [1, 256]]), t[:, 0:256])
                nc.sync.dma_start(dram_ap(E2T, c0 * W + 256, [[W, 128], [1, 256]]), t[:, 0:256])
            S2n = transpose_tiles(S2T, 512, 256, tmp, "S2n")
            T2 = tmaps(E2T, "T2_")

            XXt = []
            for (r0, nr, s2) in S2n:
                g = (G0 if r0 == 0 else G1)
                xx = tmp.tile([128, W], FP, tag=f"XX_{r0}")
                x0 = X0t[r0 // 128][2]
                nc.vector.scalar_tensor_tensor(xx[:], s2[:, 0:W], -8.0, x0[:], MUL, ADD)
                nc.vector.scalar_tensor_tensor(xx[:], g[:, 13:13 + 2 * W:2], -SQ2, xx[:], MUL, ADD)
                XXt.append((r0, nr, xx))
            XXT = transpose_tiles(XXt, 256, 512, tmp, "XXT")

            if debug:
                for (r0, nr, t) in S2n:
                    nc.sync.dma_start(dbg["dS2n"].ap()[r0:r0 + nr, :], t[:, 0:W])
                for (r0, nr, t) in XXt:
                    nc.sync.dma_start(dbg["dXX"].ap()[r0:r0 + nr, :], t[:])
            for (s0, ns, t) in XXT:
                for (k, pcol0, a0, cnt) in [(0, 16, 0, 256), (1, 272, 0, 256), (-1, 4, 244, 12), (2, 528, 0, 10)]:
                    qb0 = (280 - 256 * k + s0) % 512
                    runs = [(0, qb0, min(ns, 512 - qb0))]
                    if runs[0][2] < ns:
                        runs.append((runs[0][2], 0, ns - runs[0][2]))
                    for (srow, qs, rl) in runs:
                        for qc in (qs, qs + 512):
                            tlo = max(0, 14 - qc)
                            thi = min(rl, NEX - qc)
                            if tlo < thi:
                                nc.sync.dma_start(
                                    dram_ap(EXXT, (qc + tlo) * WEX + pcol0, [[WEX, thi - tlo], [1, cnt]]),
                                    t[srow + tlo:srow + thi, a0:a0 + cnt])

            # ========== stage 7: C-pair gathers (partitioned by j); S3; E0 ==========
            # H[jj, 2*ii+e] = EXXT[(j-i+280)*WEX + (i+j+e+16)], j = j0+p, i = ii-6
            def cpair_tile(j0, npart, dst):
                nc.sync.dma_start(dst, dram_ap(
                    EXXT, (j0 + 286) * WEX + j0 + 10, [[WEX + 1, npart], [-(WEX - 1), 267], [1, 2]]))

            H0 = keep.tile([128, 534], FP, tag="H0")
            H1 = keep.tile([128, 534], FP, tag="H1")
            H2 = keep.tile([11, 534], FP, tag="H2")
            cpair_tile(0, 128, H0[:])
            cpair_tile(128, 128, H1[:])
            cpair_tile(256, 5, H2[0:5, :])
            cpair_tile(-6, 6, H2[5:11, :])

            if debug:
                nc.sync.dma_start(dbg["dH0"].ap()[:, :], H0[:])
            # pass1 contracts j (partition dim of H), pass2 contracts i -> S3 natural
            S3n = conv2v(pair_specs([(128, H0), (128, H1), (11, H2)]), Bf256p_t, Bf256n_t, 256, 267)
            # P0B via strided copy + PE transpose: P0BT[b, a] = H_b[:, 12+2a]
            p0bt = []
            for bi, hgt in enumerate((H0, H1)):
                cp = tmp.tile([128, 256], FP, tag=f"tpc_{bi}")
                nc.vector.tensor_copy(cp[:], hgt[:, 12:12 + 2 * HH:2])
                p0bt.append((bi * 128, 128, cp))
            P0Bn = transpose_tiles(p0bt, 256, 256, tmp, "p0bn")
            if debug:
                for (r0, nr, t) in S3n:
                    nc.sync.dma_start(dbg["dS3n"].ap()[r0:r0 + nr, :], t[:, 0:HH])
                for (c0, cw, t) in P0Bn:
                    nc.sync.dma_start(dbg["dP0B"].ap()[c0:c0 + cw, :], t[:])
            for ((r0, nr, s3), (_, _, p0b)) in zip(S3n, P0Bn):
                e0 = outp.tile([128, HH], FP, tag="E0w")
                nc.vector.scalar_tensor_tensor(e0[:], s3[:, 0:HH], -16.0, p0b[:, 0:HH], MUL, ADD)
                nc.vector.tensor_scalar_mul(e0[:], e0[:], 1.0 / SQ2)
                nc.sync.dma_start(E0d.ap()[r0:r0 + nr, :], e0[:])
                if debug:
                    nc.sync.dma_start(dbg["dE0"].ap()[r0:r0 + nr, :], e0[:])

            # ========== stage 8: S4 ==========
            E0p = pad_per_from_dram(E0d, HH, HH, 5, 5, 267, 267, "padp")
            S4T = conv2v(specs_of(E0p), Bf256n_t, Bf256n_t, 256, 267)
            S4n = transpose_tiles(S4T, 256, 256, keep, "S4n")

            if debug:
                dd = np_none = None
                for (r0, nr, t) in S4n:
                    nc.sync.dma_start(dbg["dS4n"].ap()[r0:r0 + nr, :], t[:, 0:HH])
            # ========== stage 9: broadcast maps ==========
            Gmaps = {k: [] for k in "ABCE"}
            for ti in range(2):
                r0 = ti * 128
                s3 = S3n[ti][2]
                s4 = S4n[ti][2]
                t1e = T1[0][ti][2]
                t1o = T1[1][ti][2]
                t2e = T2[0][ti][2]
                t2o = T2[1][ti][2]
                me = Mp[ti][2]
                mo = Mp[2 + ti][2]
                ga = keep.tile([128, HH], FP, tag=f"GA_{r0}")
                nc.vector.tensor_scalar_mul(ga[:], s3[:, 0:HH], -1.0 / SQ2)
                nc.vector.scalar_tensor_tensor(ga[:], t1e[:, 0:HH], -0.5, ga[:], MUL, ADD)
                nc.vector.scalar_tensor_tensor(ga[:], me[:, 0:W:2], -0.5, ga[:], MUL, ADD)
                gb = keep.tile([128, HH], FP, tag=f"GB_{r0}")
                nc.vector.scalar_tensor_tensor(gb[:], s4[:, 0:HH], -1.0, t1o[:, 0:HH], MUL, ADD)
                nc.vector.scalar_tensor_tensor(gb[:], mo[:, 1:W:2], 1.0, gb[:], MUL, ADD)
                gc = keep.tile([128, HH], FP, tag=f"GC_{r0}")
                nc.vector.tensor_scalar_mul(gc[:], s3[:, 0:HH], -1.0 / SQ2)
                nc.vector.scalar_tensor_tensor(gc[:], t2e[:, 0:HH], -1.0 / SQ2, gc[:], MUL, ADD)
                nc.vector.scalar_tensor_tensor(gc[:], me[:, 1:W:2], 1.0, gc[:], MUL, ADD)
                ge = keep.tile([128, HH], FP, tag=f"GE_{r0}")
                nc.vector.tensor_scalar_mul(ge[:], s4[:, 0:HH], -1.0)
                nc.vector.scalar_tensor_tensor(ge[:], t2o[:, 0:HH], SQ2, ge[:], MUL, ADD)
                nc.vector.scalar_tensor_tensor(ge[:, 0:HH - 1], mo[:, 2:W:2], -2.0, ge[:, 0:HH - 1], MUL, ADD)
                nc.vector.scalar_tensor_tensor(ge[:, HH - 1:HH], mo[:, 0:1], -2.0, ge[:, HH - 1:HH], MUL, ADD)
                Gmaps["A"].append(ga)
                Gmaps["B"].append(gb)
                Gmaps["C"].append(gc)
                Gmaps["E"].append(ge)

            if debug:
                for ti, r0 in ((0, 0), (1, 128)):
                    for gk, nm in (("A", "dGA"), ("B", "dGB"), ("C", "dGC"), ("E", "dGE")):
                        nc.sync.dma_start(dbg[nm].ap()[r0:r0 + 128, :], Gmaps[gk][ti][:])
            # ========== stage 10: per-channel outputs ==========
            for hhalf in range(2):
                if hhalf == 0:
                    xe, xo = xe0, xo0
                    xefree, xofree = xe0free, xo0free
                else:
                    xe, xefree = tc.tile([128, NCH * W], FP, name="xout_e_1")
                    xo, xofree = tc.tile([128, NCH * W], FP, name="xout_o_1")
                    base = hhalf * 2 * 128 * W
                    nc.sync.dma_start(xe[:], dram_ap(x_h, base, [[2 * W, 128], [H * W, NCH], [1, W]]))
                    nc.sync.dma_start(xo[:], dram_ap(x_h, base + W, [[2 * W, 128], [H * W, NCH], [1, W]]))
                ga, gb, gc, ge = (Gmaps[k][hhalf] for k in "ABCE")
                r0 = hhalf * 128
                for (onm, src, off, sc, gm, wrap) in [
                        ("out_e0lo", xe, 0, 0.5, ga, False),
                        ("out_e1lo", xo, 1, -1.0, gb, False),
                        ("out_e0hi", xe, 1, -1.0, gc, False),
                        ("out_e1hi", xo, 2, 2.0, ge, True)]:
                    wide = outp.tile([128, NCH * HH], FP, tag="owide")
                    for ch in range(NCH):
                        co = ch * W
                        wv = wide[:, ch * HH:(ch + 1) * HH]
                        if not wrap:
                            nc.vector.scalar_tensor_tensor(wv, src[:, co + off:co + W:2], sc, gm[:], MUL, ADD)
                        else:
                            nc.vector.scalar_tensor_tensor(wide[:, ch * HH:ch * HH + HH - 1],
                                                           src[:, co + 2:co + W:2], sc, gm[:, 0:HH - 1], MUL, ADD)
                            nc.vector.scalar_tensor_tensor(wide[:, ch * HH + HH - 1:ch * HH + HH],
                                                           src[:, co:co + 1], sc, gm[:, HH - 1:HH], MUL, ADD)
                    nc.sync.dma_start(
                        dram_ap(outs[onm], r0 * HH, [[HH, 128], [HH * HH, NCH], [1, HH]]),
                        wide[:])
                xofree()
                xefree()

    nc.compile()
    return nc


def kernel(x, h, g, f):
    import numpy as np
    from concourse import bass_utils
    if "nc" not in _cache:
        _cache["nc"] = _build_nc()
        _cache["mats"] = _host_mats()
    nc = _cache["nc"]
    mats = _cache["mats"]
    x = np.ascontiguousarray(np.asarray(x, np.float32))
    in_maps = []
    for i in range(NCORES):
        m = {"x": x[i]}
        m.update(mats)
        in_maps.append(m)
    res = bass_utils.run_bass_kernel_spmd(nc, in_maps, core_ids=list(range(NCORES)))

    def stack(nm):
        return np.stack([res.results[i][nm] for i in range(NCORES)], axis=0)

    return (stack("out_c"), stack("out_e1lo"), stack("out_e0lo"),
            stack("out_e1hi"), stack("out_e0hi"))
